# revision 33
# baseline (speedup 1.0000x reference)
"""Trainium2 Bass kernel: ViT transformer block with Convpass adapters.

Problem nn_CTrans_42133629173960 (dense_transformer, compute-bound).

Sharding: pure data-parallel over batch — 8 NeuronCores x 4 batches each,
no collectives. On-chip layout is feature-major ([channel, token]); the host
pre-transposes x/pos (and un-transposes the output), so the device never
runs PE transposes and every GEMM contraction sits on the partition axis.

  - LayerNorm channel-reductions are ones-matmuls on the PE in bf16
    (1/C folded into the ones), Rsqrt on ScalarE.
  - Attention: scores are computed k-major (exp'd with ScalarE); the
    softmax denominator comes from a ones-column appended to V in the
    A@V matmul (PSUM row 64); the reciprocal runs on the [1,N] row
    before the partition-broadcast. V tiles are padded to a 128 stride
    per head so the A@V weight loads use fast-weight-load.
  - QKV of batch b+1 is emitted before proj of batch b so the PE stays
    busy through the softmax-normalize tail.
  - The 3x3x3 Convpass conv runs as 9 accumulated PE matmuls over a
    zero-padded (channel*dx, z, y, x) im2col buffer.
  - FFN weights live resident in SBUF (loaded once, during the convpass1
    window) in a host-prearranged contiguous tile layout.
  - Big GEMMs run in bf16 (weights pre-cast on the host); the residual
    carrier and LayerNorm statistics stay in f32r/fp32.

Self-contained: hardcodes shapes from the problem spec.
"""

import numpy as np

import concourse.bass as bass
import concourse.tile as tile
from concourse import bacc, mybir
from concourse.bass_utils import run_bass_kernel_spmd

f32 = mybir.dt.float32
f32r = mybir.dt.float32r
bf16 = mybir.dt.bfloat16
f8 = mybir.dt.float8e4
DR = mybir.MatmulPerfMode.DoubleRow
AF = mybir.ActivationFunctionType
ALU = mybir.AluOpType

B, N, C = 32, 512, 512
H, DH = 8, 64
ADIM = 8
MLP = 4096
EPS = 1e-5
SCALE = DH ** -0.5
NCORES = 8
BPC = B // NCORES          # 4 batches per core
TOK = BPC * N              # 2048 tokens per core
P = 128
CT = C // P                # 4 channel tiles
NT = N // P                # 4 token sub-tiles per batch
MT1 = MLP // P             # 32 tiles of the FFN hidden dim
W1G = 4                    # ff_w1 resident groups (8 m-tiles each)
QSCALE = 1.702             # quick-gelu sigmoid scale
GEMM_BF16 = True           # bf16 GEMM path (weights pre-cast on host)


def _bias_tiles(nc, pool, dram_ap, n_tiles, name):
    """Load a [n_tiles*128, 1] DRAM vector as per-partition scalar tiles."""
    tiles = []
    for t in range(n_tiles):
        bt = pool.tile([P, 1], f32, name=f"{name}{t}")
        nc.sync.dma_start(bt[:], dram_ap[t * P:(t + 1) * P, :])
        tiles.append(bt)
    return tiles


def build(nc, nz, sim_gelu=False, gemm_bf16=True):
    """Emit the per-core program. nz: dict of which biases are nonzero.
    sim_gelu: replace the FFN Gelu LUT with x*sigmoid(1.702x) (CoreSim has
    no Gelu implementation; only used by the simulation tests)."""
    wdt = bf16 if gemm_bf16 else f32r
    xin = nc.dram_tensor("x", [C, TOK], f32, kind="ExternalInput").ap()
    pos = nc.dram_tensor("pos", [C, TOK], f32, kind="ExternalInput").ap()
    qkv_w = nc.dram_tensor("qkv_w", [C, 3 * C], wdt, kind="ExternalInput").ap()
    proj_w = nc.dram_tensor("proj_w", [C, C], wdt, kind="ExternalInput").ap()
    # host-prearranged fp8 DoubleRow pair layouts (see make_in_maps)
    ff_w1 = nc.dram_tensor("ff_w1", [W1G, P, CT * 8 * P], f8, kind="ExternalInput").ap()
    ff_w2 = nc.dram_tensor("ff_w2", [W1G, P, 8 * C], f8, kind="ExternalInput").ap()
    cp_down_w = [nc.dram_tensor(f"cp{i}_down_w", [C, ADIM], wdt, kind="ExternalInput").ap() for i in (1, 2)]
    cp_conv_w = [nc.dram_tensor(f"cp{i}_conv_w", [ADIM, ADIM, 3, 3, 3], wdt, kind="ExternalInput").ap() for i in (1, 2)]
    cp_up_w = [nc.dram_tensor(f"cp{i}_up_w", [ADIM, C], wdt, kind="ExternalInput").ap() for i in (1, 2)]

    def opt_vec(name, length):
        if not nz.get(name, False):
            return None
        return nc.dram_tensor(name, [length, 1], f32, kind="ExternalInput").ap()

    proj_b = opt_vec("proj_b", C)
    ff_b1 = opt_vec("ff_b1", MLP)
    ff_b2 = opt_vec("ff_b2", C)
    cp_down_b = [opt_vec(f"cp{i}_down_b", ADIM) for i in (1, 2)]
    cp_conv_b = [opt_vec(f"cp{i}_conv_b", ADIM) for i in (1, 2)]
    cp_up_b = [opt_vec(f"cp{i}_up_b", C) for i in (1, 2)]
    ln_g = [opt_vec("ln1_g", C), opt_vec("ln2_g", C)]
    ln_b = [opt_vec("ln1_b", C), opt_vec("ln2_b", C)]

    out = nc.dram_tensor("out", [C, TOK], f32, kind="ExternalOutput").ap()

    with tile.TileContext(nc) as tc:
        _build_tc(nc, tc, dict(
            xin=xin, pos=pos, qkv_w=qkv_w, proj_w=proj_w, ff_w1=ff_w1,
            ff_w2=ff_w2, cp_down_w=cp_down_w, cp_conv_w=cp_conv_w,
            cp_up_w=cp_up_w, proj_b=proj_b, ff_b1=ff_b1, ff_b2=ff_b2,
            cp_down_b=cp_down_b, cp_conv_b=cp_conv_b, cp_up_b=cp_up_b,
            ln_g=ln_g, ln_b=ln_b, out=out), sim_gelu=sim_gelu,
            gemm_bf16=gemm_bf16)
    return nc


def _build_tc(nc, tc, t, sim_gelu=False, gemm_bf16=True):
    gdt = bf16 if gemm_bf16 else f32r
    wdma = lambda eng_out, dram: nc.sync.dma_start(eng_out, dram)
    from contextlib import ExitStack

    bslc = lambda b: slice(b * N, (b + 1) * N)

    with ExitStack() as top:
        const = top.enter_context(tc.tile_pool(name="const", bufs=1))
        ones_f32 = const.tile([P, P], f32, name="ones_f32")
        nc.vector.memset(ones_f32[:], 1.0)
        # bf16 ones scaled by 1/C: LN stat matmuls produce means directly
        oinv_f32 = const.tile([P, P], f32, name="oinv_f32")
        nc.vector.memset(oinv_f32[:], 1.0 / C)
        ones_inv = const.tile([P, P], bf16, name="ones_inv")
        nc.vector.tensor_copy(ones_inv[:], oinv_f32[:])
        zeros_f32 = const.tile([P, 800], f32, name="zeros_f32")
        nc.vector.memset(zeros_f32[:], 0.0)
        eps_t = const.tile([P, 1], f32, name="eps_t")
        nc.vector.memset(eps_t[:], EPS)

        bias_pool = top.enter_context(tc.tile_pool(name="biases", bufs=1))
        projb_sb = _bias_tiles(nc, bias_pool, t["proj_b"], CT, "projb") if t["proj_b"] is not None else None
        ffb1_sb = _bias_tiles(nc, bias_pool, t["ff_b1"], MT1, "ffb1") if t["ff_b1"] is not None else None
        ffb2_sb = _bias_tiles(nc, bias_pool, t["ff_b2"], CT, "ffb2") if t["ff_b2"] is not None else None
        lng_sb = [None, None]
        lnb_sb = [None, None]
        for i in range(2):
            if t["ln_g"][i] is not None:
                lng_sb[i] = _bias_tiles(nc, bias_pool, t["ln_g"][i], CT, f"lng{i}")
            if t["ln_b"][i] is not None:
                lnb_sb[i] = _bias_tiles(nc, bias_pool, t["ln_b"][i], CT, f"lnb{i}")
        cp_db = [None, None]
        cp_cb = [None, None]
        cp_upb = [None, None]
        for i in range(2):
            if t["cp_down_b"][i] is not None:
                db = bias_pool.tile([96, 1], f32, name=f"db_{i}")
                nc.vector.tensor_copy(db[:], zeros_f32[0:96, 0:1])
                for dxi in range(3):
                    nc.sync.dma_start(db[32 * dxi:32 * dxi + ADIM, :], t["cp_down_b"][i][:])
                cp_db[i] = db
            if t["cp_conv_b"][i] is not None:
                cb = bias_pool.tile([ADIM, 1], f32, name=f"cb_{i}")
                nc.sync.dma_start(cb[:], t["cp_conv_b"][i][:])
                cp_cb[i] = cb
            if t["cp_up_b"][i] is not None:
                cp_upb[i] = _bias_tiles(nc, bias_pool, t["cp_up_b"][i], CT, f"upb{i}")

        # convpass weights (small, persistent). The 3 x-shifts of the conv
        # live at partition blocks 32*dxi (8 rows each); unused rows are
        # zeroed so they contribute nothing to the contraction.
        cpw_pool = top.enter_context(tc.tile_pool(name="cpw", bufs=1))
        dw96_sb = [[], []]
        w96_sb = [None, None]
        upw_sb = [None, None]
        for i in range(2):
            for ct in range(CT):
                dw96_sb[i].append(cpw_pool.tile([P, 96], gdt, name=f"dw96_{i}_{ct}"))
            w96_sb[i] = cpw_pool.tile([96, 9, ADIM], gdt, name=f"w96_{i}")
            upw_sb[i] = cpw_pool.tile([ADIM, C], gdt, name=f"upw_{i}")

        def load_cpw():
            # emitted after phase 0 so these small strided DMAs never sit
            # in the sync queue ahead of the x/pos input stream
            for i in range(2):
                for ct in range(CT):
                    dw96 = dw96_sb[i][ct]
                    nc.vector.tensor_copy(dw96[:], zeros_f32[:, 0:96])
                    for dxi in range(3):
                        wdma(dw96[:, 32 * dxi:32 * dxi + ADIM],
                             t["cp_down_w"][i][ct * P:(ct + 1) * P, :])
                w96 = w96_sb[i]
                nc.vector.tensor_copy(w96[:].rearrange("p a b -> p (a b)"), zeros_f32[0:96, 0:72])
                for dxi in range(3):
                    nc.gpsimd.dma_start(
                        w96[32 * dxi:32 * dxi + ADIM, :, :],
                        t["cp_conv_w"][i][:, :, :, :, dxi].rearrange("o i dz dy -> i (dz dy) o"))
                wdma(upw_sb[i][:], t["cp_up_w"][i][:])

        # persistent activations; xT doubles as the residual carrier
        pool_xT = top.enter_context(tc.tile_pool(name="xT", bufs=1))
        xT = [pool_xT.tile([P, TOK], f32r, name=f"xT{ct}") for ct in range(CT)]
        pool_hT = top.enter_context(tc.tile_pool(name="hT", bufs=1))
        hT = [pool_hT.tile([P, TOK], gdt, name=f"hT{ct}") for ct in range(CT)]
        x1T = xT

        # ---- attention weights: loaded after batch 0 of x/pos (below) ----
        att_es = ExitStack()
        wpool = att_es.enter_context(tc.tile_pool(name="attw", bufs=1))
        qkvw_sb = [wpool.tile([P, 3 * C], gdt, name=f"qkvw{ct}") for ct in range(CT)]
        projw_sb = [wpool.tile([P, C], gdt, name=f"projw{ct}") for ct in range(CT)]

        # ---- LayerNorm: stats via bf16 ones-matmuls (1/C folded in),
        #      Rsqrt on ScalarE; the normalize itself reads full-precision x
        def layer_norm_one(src, dst, which, b, lnps, lnsb, psname="ln_s"):
            xb = []
            sq = []
            for ct in range(CT):
                xb_t = lnsb.tile([P, N], bf16, name="ln_xb", bufs=3)
                nc.vector.tensor_copy(xb_t[:], src[ct][:, bslc(b)])
                xb.append(xb_t)
                sq_t = lnsb.tile([P, N], bf16, name="ln_sq", bufs=3)
                nc.scalar.activation(sq_t[:], src[ct][:, bslc(b)], AF.Square)
                sq.append(sq_t)
            m = lnps.tile([P, N], f32, name=psname)
            for ct in range(CT):
                nc.tensor.matmul(m[:], ones_inv[:], xb[ct][:],
                                 start=(ct == 0), stop=(ct == CT - 1))
            e2 = lnps.tile([P, N], f32, name=psname)
            for ct in range(CT):
                nc.tensor.matmul(e2[:], ones_inv[:], sq[ct][:],
                                 start=(ct == 0), stop=(ct == CT - 1))
            # stats on the [1,N] row (rows are identical), then broadcast.
            # rstd = exp(-0.5*ln(var+eps)): InstReciprocal costs ~8 cyc/elem
            # on DVE regardless of partition count, so use ScalarE LUTs.
            msq1 = lnsb.tile([1, N], f32, name="ln_msq1", bufs=1)
            nc.scalar.activation(msq1[:], m[0:1, :], AF.Square)
            var1 = lnsb.tile([1, N], f32, name="ln_var1", bufs=1)
            nc.vector.tensor_sub(var1[:], e2[0:1, :], msq1[:])
            lnv1 = lnsb.tile([1, N], f32, name="ln_lnv1", bufs=1)
            nc.scalar.activation(lnv1[:], var1[:], AF.Ln, bias=eps_t[0:1, :])
            rstd1 = lnsb.tile([1, N], f32, name="ln_rstd1", bufs=1)
            nc.scalar.activation(rstd1[:], lnv1[:], AF.Exp, scale=-0.5)
            mr1 = lnsb.tile([1, N], f32, name="ln_mr1", bufs=1)
            nc.vector.tensor_mul(mr1[:], m[0:1, :], rstd1[:])
            rstd = lnsb.tile([P, N], f32, name="ln_rstd", bufs=2)
            nc.gpsimd.partition_broadcast(rstd[:], rstd1[:])
            mr = lnsb.tile([P, N], f32, name="ln_mr", bufs=1)
            nc.gpsimd.partition_broadcast(mr[:], mr1[:])
            for ct in range(CT):
                eng = nc.vector if ct < 2 else nc.gpsimd
                tmp = lnsb.tile([P, N], f32, name=f"ln_tmp{ct % 2}_{ct // 2}", bufs=1)
                eng.tensor_mul(tmp[:], src[ct][:, bslc(b)], rstd[:])
                eng.tensor_sub(dst[ct][:, bslc(b)], tmp[:], mr[:])
                if lng_sb[which] is not None or lnb_sb[which] is not None:
                    g = lng_sb[which][ct][:] if lng_sb[which] is not None else 1.0
                    bb = lnb_sb[which][ct][:] if lnb_sb[which] is not None else 0.0
                    nc.vector.tensor_scalar(dst[ct][:, bslc(b)], dst[ct][:, bslc(b)],
                                            g, bb, op0=ALU.mult, op1=ALU.add)

        def layer_norm(src, dst, which, es):
            lnps = es.enter_context(tc.tile_pool(name=f"ln{which}ps", bufs=2, space="PSUM"))
            lnsb = es.enter_context(tc.tile_pool(name=f"ln{which}sb", bufs=3))
            for b in range(BPC):
                layer_norm_one(src, dst, which, b, lnps, lnsb)

        # ---- Convpass, batch-staged so the PE never waits on the
        #      qgelu/im2col chain of the batch it is about to convolve ----
        def convpass_all(i, src, fold, es, after_batch=None):
            csb = es.enter_context(tc.tile_pool(name=f"cp{i}sb", bufs=1))
            dnps = es.enter_context(tc.tile_pool(name=f"cp{i}dn", bufs=4, space="PSUM"))
            cvps = es.enter_context(tc.tile_pool(name=f"cp{i}cv", bufs=2, space="PSUM"))
            upps = es.enter_context(tc.tile_pool(name=f"cp{i}up", bufs=2, space="PSUM"))
            ims, pts = [], []
            for b in range(BPC):
                d_ps = dnps.tile([96, N], f32, name="cp_dps")
                for ct in range(CT):
                    nc.tensor.matmul(d_ps[:], dw96_sb[i][ct][:], src[ct][:, bslc(b)],
                                     start=(ct == 0), stop=(ct == CT - 1))
                if cp_db[i] is not None:
                    dz = csb.tile([96, N], f32, name="cp_dz", bufs=4)
                    nc.vector.tensor_scalar_add(dz[:], d_ps[:], cp_db[i][:])
                    d_in = dz
                else:
                    d_in = d_ps
                sg = csb.tile([96, N], f32, name="cp_sg", bufs=4)
                nc.scalar.activation(sg[:], d_in[:], AF.Sigmoid, scale=QSCALE)
                d96 = csb.tile([96, N], gdt, name="cp_d96", bufs=4)
                nc.vector.tensor_mul(d96[:], d_in[:], sg[:])
                # fresh per-batch im2col buffer; zero it, then write the
                # interior. Block dxi holds in[..., x + dxi - 1].
                im96 = csb.tile([96, 10, 10, 8], gdt, name="cp_im96", bufs=4)
                nc.vector.tensor_copy(im96[:].rearrange("p a b c -> p (a b c)"),
                                      zeros_f32[0:96, 0:800])
                dv = d96[:].rearrange("p (z y x) -> p z y x", z=8, y=8)
                nc.vector.tensor_copy(im96[0:8, 1:9, 1:9, 1:8], dv[0:8, :, :, 0:7])
                nc.vector.tensor_copy(im96[32:40, 1:9, 1:9, 0:8], dv[32:40, :, :, 0:8])
                nc.vector.tensor_copy(im96[64:72, 1:9, 1:9, 0:7], dv[64:72, :, :, 1:8])
                ims.append(im96)
            for b in range(BPC):
                cv_ps = cvps.tile([ADIM, N], f32, name="cp_cvps")
                cv_view = cv_ps[:].rearrange("p (z y x) -> p z y x", z=8, y=8)
                for tap in range(9):
                    dzz, dyy = tap // 3, tap % 3
                    nc.tensor.matmul(cv_view, w96_sb[i][:, tap, :],
                                     ims[b][0:96, dzz:dzz + 8, dyy:dyy + 8, 0:8],
                                     start=(tap == 0), stop=(tap == 8))
                if cp_cb[i] is not None:
                    cz = csb.tile([ADIM, N], f32, name="cp_cz", bufs=4)
                    nc.vector.tensor_scalar_add(cz[:], cv_ps[:], cp_cb[i][:])
                    c_in = cz
                else:
                    c_in = cv_ps
                sg2 = csb.tile([ADIM, N], f32, name="cp_sg2", bufs=4)
                nc.scalar.activation(sg2[:], c_in[:], AF.Sigmoid, scale=QSCALE)
                pt = csb.tile([ADIM, N], gdt, name="cp_pt", bufs=4)
                nc.vector.tensor_mul(pt[:], c_in[:], sg2[:])
                pts.append(pt)
            for b in range(BPC):
                for ct in range(CT):
                    up_ps = upps.tile([P, N], f32, name="cp_upps")
                    nc.tensor.matmul(up_ps[:], upw_sb[i][:, ct * P:(ct + 1) * P],
                                     pts[b][:], start=True, stop=True)
                    fold(b, ct, up_ps)
                if after_batch is not None:
                    after_batch(b, upps)

        # ---- Phase 2: attention (+ proj, x1 = x + proj in place).
        #      QKV of batch b+1 is emitted before proj of batch b. ----
        with ExitStack() as esw:
            qk_pool = esw.enter_context(tc.tile_pool(name="qksb", bufs=2))
            v_pool = esw.enter_context(tc.tile_pool(name="vsb", bufs=1))
            e_pool = esw.enter_context(tc.tile_pool(name="esb", bufs=1))
            a_pool = esw.enter_context(tc.tile_pool(name="acsb", bufs=1))
            n_pool = esw.enter_context(tc.tile_pool(name="nsb", bufs=2))

            qkps = esw.enter_context(tc.tile_pool(name="qkps", bufs=2, space="PSUM"))
            scps = esw.enter_context(tc.tile_pool(name="scps", bufs=2, space="PSUM"))
            avps = esw.enter_context(tc.tile_pool(name="avps", bufs=4, space="PSUM"))
            p0sb = esw.enter_context(tc.tile_pool(name="p0", bufs=6))
            ln1sb = esw.enter_context(tc.tile_pool(name="ln0sb", bufs=3))

            def phase0_ln1(b):
                # load+add batch b, then LN1(b); stats matmuls share the
                # qk_ps PSUM ring to stay within the 8-bank budget
                for ct in range(CT):
                    xt = p0sb.tile([P, N], f32, name="xt_in")
                    pt = p0sb.tile([P, N], f32, name="pt_in")
                    nc.sync.dma_start(xt[:], t["xin"][ct * P:(ct + 1) * P, bslc(b)])
                    nc.sync.dma_start(pt[:], t["pos"][ct * P:(ct + 1) * P, bslc(b)])
                    (nc.vector if ct < 2 else nc.gpsimd).tensor_add(
                        xT[ct][:, bslc(b)], xt[:], pt[:])
                if b == 0:
                    for ct in range(CT):
                        wdma(qkvw_sb[ct][:], t["qkv_w"][ct * P:(ct + 1) * P, :])
                    for ct in range(CT):
                        wdma(projw_sb[ct][:], t["proj_w"][ct * P:(ct + 1) * P, :])
                layer_norm_one(xT, hT, 0, b, qkps, ln1sb, psname="qk_ps")

            def emit_qkv(b):
                qk_sb = []
                v_sb = []
                for mt in range(8):
                    qk_ps = qkps.tile([P, N], f32, name="qk_ps")
                    for ct in range(CT):
                        nc.tensor.matmul(qk_ps[:], qkvw_sb[ct][:, mt * P:(mt + 1) * P],
                                         hT[ct][:, bslc(b)],
                                         start=(ct == 0), stop=(ct == CT - 1))
                    if mt < 4:
                        # Q side: per-head tiles, other head's rows zeroed —
                        # the zeros make the full-width K tiles contract
                        # correctly at K=128 (partial-K matmuls lose FWL)
                        for hh in range(2):
                            qp = qk_pool.tile([P, N], gdt, name=f"q{mt}_{hh}")
                            if b < 2:  # bufs=2 slots keep their zero half
                                nc.vector.tensor_copy(
                                    qp[DH - hh * DH:P - hh * DH, :],
                                    zeros_f32[0:DH, 0:N])
                            nc.vector.tensor_copy(
                                qp[hh * DH:(hh + 1) * DH, :],
                                qk_ps[hh * DH:(hh + 1) * DH, :])
                            qk_sb.append((mt, hh, qp))
                    else:
                        # K side: one full-width copy per m-tile
                        qk_t = qk_pool.tile([P, N], gdt, name=f"k{mt}")
                        nc.vector.tensor_copy(qk_t[:], qk_ps[:])
                        qk_sb.append(qk_t)
                for s in range(NT):
                    v_ps = qkps.tile([P, C], f32, name="qk_ps")
                    for ct in range(CT):
                        nc.tensor.matmul(v_ps[:], hT[ct][:, b * N + s * P: b * N + (s + 1) * P],
                                         qkvw_sb[ct][:, 2 * C:3 * C],
                                         start=(ct == 0), stop=(ct == CT - 1))
                    # per-head stride 128 (full lhsT width => FWL); col DH
                    # holds the softmax-denominator ones, cols DH+1.. zeros
                    v_t = v_pool.tile([P, H * P], gdt, name=f"vt{s}")
                    vv = v_t[:].rearrange("p (h e) -> p h e", h=H)
                    nc.vector.tensor_copy(
                        vv[:, :, 0:DH],
                        v_ps[:].rearrange("p (h d) -> p h d", h=H))
                    nc.vector.tensor_copy(
                        vv[:, :, DH:DH + 1],
                        ones_f32[:, 0:H].rearrange("p (h o) -> p h o", o=1))
                    if b == 0:  # bufs=1 slot keeps its zero pad after first init
                        nc.vector.tensor_copy(
                            vv[:, :, DH + 1:P],
                            zeros_f32[:, 0:H * (P - DH - 1)].rearrange("p (h o) -> p h o", h=H))
                    v_sb.append(v_t)
                return qk_sb, v_sb

            def emit_scores(qk_sb, hp):
                e_sb = {}
                for hh in range(2):
                    for kt in range(NT):
                        sc_ps = scps.tile([P, N], f32, name="sc_ps", bufs=2)
                        nc.tensor.matmul(sc_ps[:],
                                         qk_sb[8 + hp][:, kt * P:(kt + 1) * P],
                                         qk_sb[2 * hp + hh][2][:],
                                         start=True, stop=True)
                        e_t = e_pool.tile([P, N], gdt, name=f"e{hp % 2}_{hh}_{kt}")
                        nc.scalar.activation(e_t[:], sc_ps[:], AF.Exp, scale=SCALE)
                        e_sb[(hh, kt)] = e_t
                return e_sb

            def emit_avmm(v_sb, den4, hp, e_sb, dk):
                # A@V matmuls for one pair; denominator rows land 32 apart
                # in the group's shared den4 tile
                avs = []
                for hh in range(2):
                    h = 2 * hp + hh
                    av_ps = avps.tile([P, N], f32, name="av_ps")
                    for kt in range(NT):
                        nc.tensor.matmul(av_ps[:],
                                         v_sb[kt][:, h * P:(h + 1) * P],
                                         e_sb[(hh, kt)][:],
                                         start=(kt == 0), stop=(kt == NT - 1))
                    avs.append(av_ps)
                    k = dk + hh
                    nc.vector.tensor_copy(den4[32 * k:32 * k + 1, :],
                                          av_ps[DH:DH + 1, :])
                return avs

            def emit_norm(ac_sb, g, den4, avs):
                # one InstReciprocal per 4 heads (cost ~8 cyc per free-size
                # element regardless of the partition count)
                rcp4 = n_pool.tile([97, N], f32, name="rcp4")
                nc.vector.reciprocal(rcp4[:], den4[:])
                for k in range(4):
                    h = 4 * g + k
                    if k == 0:
                        rsrc = rcp4[0:1, :]
                    else:
                        r1 = n_pool.tile([1, N], f32, name=f"r1_{k}")
                        nc.vector.tensor_copy(r1[:], rcp4[32 * k:32 * k + 1, :])
                        rsrc = r1[:]
                    rn = n_pool.tile([DH, N], f32, name="rn")
                    nc.gpsimd.partition_broadcast(rn[:], rsrc)
                    orow = (h % 2) * DH
                    nc.vector.tensor_tensor(
                        ac_sb[h // 2][orow:orow + DH, :],
                        avs[k][0:DH, :], rn[:], op=ALU.mult)

            def emit_proj(b, ac_sb):
                for ct in range(CT):
                    pr_ps = avps.tile([P, N], f32, name="av_ps")
                    for kt in range(CT):
                        nc.tensor.matmul(pr_ps[:], projw_sb[kt][:, ct * P:(ct + 1) * P],
                                         ac_sb[kt][:], start=(kt == 0), stop=(kt == CT - 1))
                    if projb_sb is not None:
                        prb = n_pool.tile([P, N], f32, name="prb")
                        nc.vector.tensor_scalar_add(prb[:], pr_ps[:], projb_sb[ct][:])
                        nc.vector.tensor_add(x1T[ct][:, bslc(b)], xT[ct][:, bslc(b)], prb[:])
                    else:
                        nc.vector.tensor_add(x1T[ct][:, bslc(b)], xT[ct][:, bslc(b)], pr_ps[:])

            phase0_ln1(0)
            qkv_cur = emit_qkv(0)
            for b in range(BPC):
                qk_sb, v_sb = qkv_cur
                ac_sb = [a_pool.tile([P, N], gdt, name=f"ac{ct}") for ct in range(CT)]
                # software-pipelined by pair-group: scores/exp of group 1
                # are emitted before the A@V of group 0, so the in-order PE
                # always has score matmuls to run while ScalarE exps.
                e0 = emit_scores(qk_sb, 0)
                e1 = emit_scores(qk_sb, 1)
                den_a = n_pool.tile([97, N], f32, name="den_a")
                avs_a = emit_avmm(v_sb, den_a, 0, e0, 0)
                avs_a += emit_avmm(v_sb, den_a, 1, e1, 2)
                if b + 1 < BPC:
                    phase0_ln1(b + 1)
                e2 = emit_scores(qk_sb, 2)
                e3 = emit_scores(qk_sb, 3)
                emit_norm(ac_sb, 0, den_a, avs_a)
                den_b = n_pool.tile([97, N], f32, name="den_b")
                avs_b = emit_avmm(v_sb, den_b, 2, e2, 0)
                avs_b += emit_avmm(v_sb, den_b, 3, e3, 2)
                if b + 1 < BPC:
                    qkv_cur = emit_qkv(b + 1)
                emit_norm(ac_sb, 1, den_b, avs_b)
                emit_proj(b, ac_sb)
        att_es.close()
        load_cpw()

        # ---- FFN weights: resident, loaded during the convpass1 window ----
        ffw_es = ExitStack()
        ffw_pool = ffw_es.enter_context(tc.tile_pool(name="ffw", bufs=1))
        w1_res = []
        w2_res = []
        for g in range(W1G):
            w1t = ffw_pool.tile([P, 2, 2, 8, P], f8, name=f"w1_{g}")
            wdma(w1t[:].rearrange("p a b c d -> p (a b c d)"), t["ff_w1"][g])
            w1_res.append(w1t)
        for g in range(W1G):
            w2t = ffw_pool.tile([P, 4, 2, C], f8, name=f"w2_{g}")
            wdma(w2t[:].rearrange("p a b c -> p (a b c)"), t["ff_w2"][g])
            w2_res.append(w2t)

        def w1_ap(mt, ctp):
            # [K=128, 2, M=128] DoubleRow stationary operand
            return w1_res[mt // 8][:, ctp, :, mt % 8, :]

        def w2_ap(mtp, ct):
            return w2_res[(2 * mtp) // 8][:, mtp % 4, :, ct * P:(ct + 1) * P]

        # LN2 output aliases hT; h8 (fp8 pair copy for the FFN GEMMs)
        # is cast at FFN phase start
        h2T = hT
        h8_pool = ffw_es.enter_context(tc.tile_pool(name="h8", bufs=1))
        h8 = [h8_pool.tile([P, 2, TOK], f8, name=f"h8_{cp}") for cp in range(2)]

        # ---- Phase 2b: convpass1, folded into x1 ----
        with ExitStack() as escp1:
            def fold1(b, ct, up_ps):
                if cp_upb[0] is not None:
                    ub = escp1_sb.tile([P, N], f32, name="upb_t", bufs=2)
                    nc.vector.tensor_scalar_add(ub[:], up_ps[:], cp_upb[0][ct][:])
                    nc.vector.tensor_add(x1T[ct][:, bslc(b)], x1T[ct][:, bslc(b)], ub[:])
                else:
                    nc.vector.tensor_add(x1T[ct][:, bslc(b)], x1T[ct][:, bslc(b)], up_ps[:])
            escp1_sb = escp1.enter_context(tc.tile_pool(name="cp1fold", bufs=1))
            convpass_all(0, hT, fold1, escp1)

        # ---- Phase 3: LN2 (h2 overwrites hT; convpass1 is done with it) ----
        with ExitStack() as es2:
            lnps2 = es2.enter_context(tc.tile_pool(name="ln2ps", bufs=2, space="PSUM"))
            lnsb2 = es2.enter_context(tc.tile_pool(name="ln2sb", bufs=3))
            for b in range(BPC):
                layer_norm_one(x1T, h2T, 1, b, lnps2, lnsb2, psname="ln2_ps")

        # ---- Phase 4: convpass2, folded into x1T ----
        with ExitStack() as escp2:
            def fold2(b, ct, up_ps):
                if cp_upb[1] is not None:
                    ub = escp2_sb.tile([P, N], f32, name="upb2_t", bufs=2)
                    nc.vector.tensor_scalar_add(ub[:], up_ps[:], cp_upb[1][ct][:])
                    nc.vector.tensor_add(x1T[ct][:, bslc(b)], x1T[ct][:, bslc(b)], ub[:])
                else:
                    nc.vector.tensor_add(x1T[ct][:, bslc(b)], x1T[ct][:, bslc(b)], up_ps[:])
            escp2_sb = escp2.enter_context(tc.tile_pool(name="cp2fold", bufs=1))
            convpass_all(1, h2T, fold2, escp2)

        # ---- Phase 5: fused FFN per batch + residual + store ----
        # f2 accumulates in PSUM across all 32 hidden m-tiles; f1 of tile m+1
        # is emitted before f2 of tile m so the PE never waits on the gelu.
        with tc.tile_pool(name="gmsb", bufs=3) as gmsb, \
             tc.tile_pool(name="outsb", bufs=4) as outsb, \
             tc.tile_pool(name="f1ps", bufs=3, space="PSUM") as f1ps, \
             tc.tile_pool(name="f2ps", bufs=1, space="PSUM") as f2ps:
            for b in range(BPC):
                for ct in range(CT):
                    nc.vector.tensor_copy(h8[ct // 2][:, ct % 2, bslc(b)],
                                          h2T[ct][:, bslc(b)])
            for b in range(BPC):
                f2acc = [f2ps.tile([P, N], f32, name=f"f2acc{ct}") for ct in range(CT)]

                def emit_f2(mtp, g8t, f2acc=f2acc):
                    for ct in range(CT):
                        nc.tensor.matmul(f2acc[ct][:], w2_ap(mtp, ct), g8t[:],
                                         start=(mtp == 0), stop=(mtp == MT1 // 2 - 1),
                                         perf_mode=DR)

                prev_g8 = None
                for mtp in range(MT1 // 2):
                    g8t = gmsb.tile([P, 2, N], f8, name="g8")
                    for jj in range(2):
                        mt = 2 * mtp + jj
                        f1_ps = f1ps.tile([P, N], f32, name="f1_ps")
                        for ctp in range(2):
                            nc.tensor.matmul(f1_ps[:], w1_ap(mt, ctp),
                                             h8[ctp][:, :, bslc(b)],
                                             start=(ctp == 0), stop=(ctp == 1),
                                             perf_mode=DR)
                        bias = ffb1_sb[mt][:] if ffb1_sb is not None else 0.0
                        if sim_gelu:
                            fsg = gmsb.tile([P, N], f32, name="fsg")
                            nc.scalar.activation(fsg[:], f1_ps[:], AF.Sigmoid,
                                                 scale=QSCALE, bias=bias)
                            nc.vector.tensor_mul(g8t[:, jj, :], f1_ps[:], fsg[:])
                        else:
                            nc.scalar.activation(g8t[:, jj, :], f1_ps[:], AF.Gelu,
                                                 bias=bias)
                    if prev_g8 is not None:
                        emit_f2(mtp - 1, prev_g8)
                    prev_g8 = g8t
                emit_f2(MT1 // 2 - 1, prev_g8)

                for ct in range(CT):
                    ofm = outsb.tile([P, N], f32, name="ofm")
                    if ffb2_sb is not None:
                        f2b = outsb.tile([P, N], f32, name="f2b")
                        nc.vector.tensor_scalar_add(f2b[:], f2acc[ct][:], ffb2_sb[ct][:])
                        nc.vector.tensor_add(ofm[:], x1T[ct][:, bslc(b)], f2b[:])
                    else:
                        nc.vector.tensor_add(ofm[:], x1T[ct][:, bslc(b)], f2acc[ct][:])
                    nc.sync.dma_start(
                        t["out"][ct * P:(ct + 1) * P, bslc(b)], ofm[:])
        ffw_es.close()


_CACHE = {}


def _get_compiled(nz_key, nz):
    if nz_key not in _CACHE:
        nc = bacc.Bacc("TRN2", target_bir_lowering=False, debug=False,
                       num_devices=NCORES)
        build(nc, nz)
        nc.compile()
        _CACHE[nz_key] = nc
    return _CACHE[nz_key]


def input_flags(inputs):
    nz = {}
    vec_names = ["proj_b", "ff_b1", "ff_b2", "cp1_down_b", "cp1_conv_b",
                 "cp1_up_b", "cp2_down_b", "cp2_conv_b", "cp2_up_b",
                 "ln1_b", "ln2_b"]
    for n in vec_names:
        nz[n] = bool(np.any(np.asarray(inputs[n]) != 0.0))
    nz["ln1_g"] = not bool(np.all(np.asarray(inputs["ln1_g"]) == 1.0))
    nz["ln2_g"] = not bool(np.all(np.asarray(inputs["ln2_g"]) == 1.0))
    return nz


def make_in_maps(inputs, nz):
    import ml_dtypes
    wnp = ml_dtypes.bfloat16 if GEMM_BF16 else np.float32
    x = np.asarray(inputs["x"], dtype=np.float32)
    pos = np.asarray(inputs["pos"], dtype=np.float32)
    common = {}
    for n in ["qkv_w", "proj_w"]:
        common[n] = np.ascontiguousarray(np.asarray(inputs[n], np.float32).astype(wnp))
    f8np = ml_dtypes.float8_e4m3fn
    # ff_w1 fp8 DoubleRow layout: [g, p, ctp, j, mtj, m], ct = 2*ctp + j
    w1 = np.asarray(inputs["ff_w1"], np.float32).reshape(2, 2, P, W1G, 8, P)
    common["ff_w1"] = np.ascontiguousarray(
        w1.transpose(3, 2, 0, 1, 4, 5).reshape(W1G, P, CT * 8 * P).astype(f8np))
    # ff_w2 fp8 DoubleRow layout: [g, p, mtpg, jj, m], mt = 2*mtp + jj
    w2 = np.asarray(inputs["ff_w2"], np.float32).reshape(W1G, 4, 2, P, C)
    common["ff_w2"] = np.ascontiguousarray(
        w2.transpose(0, 3, 1, 2, 4).reshape(W1G, P, 8 * C).astype(f8np))
    for i in (1, 2):
        for n in (f"cp{i}_down_w", f"cp{i}_conv_w", f"cp{i}_up_w"):
            common[n] = np.ascontiguousarray(np.asarray(inputs[n], np.float32).astype(wnp))
    for n, flag in nz.items():
        if flag:
            common[n] = np.ascontiguousarray(
                np.asarray(inputs[n], np.float32)).reshape(-1, 1)
    in_maps = []
    for c in range(NCORES):
        m = dict(common)
        m["x"] = np.ascontiguousarray(
            x[c * BPC:(c + 1) * BPC].transpose(2, 0, 1).reshape(C, TOK))
        m["pos"] = np.ascontiguousarray(
            pos[c * BPC:(c + 1) * BPC].transpose(2, 0, 1).reshape(C, TOK))
        in_maps.append(m)
    return in_maps


def kernel(**inputs):
    nz = input_flags(inputs)
    nz_key = tuple(sorted((k, v) for k, v in nz.items()))
    nc = _get_compiled(nz_key, nz)
    in_maps = make_in_maps(inputs, nz)
    res = run_bass_kernel_spmd(nc, in_maps, core_ids=list(range(NCORES)))
    out = np.concatenate(
        [res.results[c]["out"].reshape(C, BPC, N).transpose(1, 2, 0)
         for c in range(NCORES)], axis=0)
    return np.ascontiguousarray(out.astype(np.float32))


if __name__ == "__main__":
    # quick self-build check (no run)
    nc = bacc.Bacc("TRN2", target_bir_lowering=False, debug=False, num_devices=NCORES)
    build(nc, {})
    nc.compile()
    print("built + compiled OK; instructions:",
          sum(len(bb.instructions) for bb in nc.main_func.blocks))


# revision 34
# speedup vs baseline: 1.3825x; 1.3825x over previous
"""Trainium2 Bass kernel: ViT transformer block with Convpass adapters.

Problem nn_CTrans_42133629173960 (dense_transformer, compute-bound).

Sharding: pure data-parallel over batch — 8 NeuronCores x 4 batches each,
no collectives. On-chip layout is feature-major ([channel, token]); the host
pre-transposes x/pos (and un-transposes the output), so the device never
runs PE transposes and every GEMM contraction sits on the partition axis.

  - LayerNorm channel-reductions are ones-matmuls on the PE in bf16
    (1/C folded into the ones), Rsqrt on ScalarE.
  - Attention: scores are computed k-major (exp'd with ScalarE); the
    softmax denominator comes from a ones-column appended to V in the
    A@V matmul (PSUM row 64); the reciprocal runs on the [1,N] row
    before the partition-broadcast. V tiles are padded to a 128 stride
    per head so the A@V weight loads use fast-weight-load.
  - QKV of batch b+1 is emitted before proj of batch b so the PE stays
    busy through the softmax-normalize tail.
  - The 3x3x3 Convpass conv runs as 9 accumulated PE matmuls over a
    zero-padded (channel*dx, z, y, x) im2col buffer.
  - FFN weights live resident in SBUF (loaded once, during the convpass1
    window) in a host-prearranged contiguous tile layout.
  - Big GEMMs run in bf16 (weights pre-cast on the host); the residual
    carrier and LayerNorm statistics stay in f32r/fp32.

Self-contained: hardcodes shapes from the problem spec.
"""

import numpy as np

import concourse.bass as bass
import concourse.tile as tile
from concourse import bacc, mybir
from concourse.bass_utils import run_bass_kernel_spmd

f32 = mybir.dt.float32
f32r = mybir.dt.float32r
bf16 = mybir.dt.bfloat16
f8 = mybir.dt.float8e4
DR = mybir.MatmulPerfMode.DoubleRow
AF = mybir.ActivationFunctionType
ALU = mybir.AluOpType

B, N, C = 32, 512, 512
H, DH = 8, 64
ADIM = 8
MLP = 4096
EPS = 1e-5
SCALE = DH ** -0.5
NCORES = 8
BPC = B // NCORES          # 4 batches per core
TOK = BPC * N              # 2048 tokens per core
P = 128
CT = C // P                # 4 channel tiles
NT = N // P                # 4 token sub-tiles per batch
MT1 = MLP // P             # 32 tiles of the FFN hidden dim
W1G = 4                    # ff_w1 resident groups (8 m-tiles each)
QSCALE = 1.702             # quick-gelu sigmoid scale
GEMM_BF16 = True           # bf16 GEMM path (weights pre-cast on host)


def _bias_tiles(nc, pool, dram_ap, n_tiles, name):
    """Load a [n_tiles*128, 1] DRAM vector as per-partition scalar tiles."""
    tiles = []
    for t in range(n_tiles):
        bt = pool.tile([P, 1], f32, name=f"{name}{t}")
        nc.sync.dma_start(bt[:], dram_ap[t * P:(t + 1) * P, :])
        tiles.append(bt)
    return tiles


def build(nc, nz, sim_gelu=False, gemm_bf16=True):
    """Emit the per-core program. nz: dict of which biases are nonzero.
    sim_gelu: replace the FFN Gelu LUT with x*sigmoid(1.702x) (CoreSim has
    no Gelu implementation; only used by the simulation tests)."""
    wdt = bf16 if gemm_bf16 else f32r
    xin = nc.dram_tensor("x", [C, TOK], f32, kind="ExternalInput").ap()
    pos = nc.dram_tensor("pos", [C, TOK], f32, kind="ExternalInput").ap()
    qkv_w = nc.dram_tensor("qkv_w", [C, 3 * C], wdt, kind="ExternalInput").ap()
    proj_w = nc.dram_tensor("proj_w", [C, C], wdt, kind="ExternalInput").ap()
    # host-prearranged fp8 DoubleRow pair layouts (see make_in_maps)
    ff_w1 = nc.dram_tensor("ff_w1", [W1G, P, CT * 8 * P], f8, kind="ExternalInput").ap()
    ff_w2 = nc.dram_tensor("ff_w2", [W1G, P, 8 * C], f8, kind="ExternalInput").ap()
    cp_down_w = [nc.dram_tensor(f"cp{i}_down_w", [C, ADIM], wdt, kind="ExternalInput").ap() for i in (1, 2)]
    cp_conv_w = [nc.dram_tensor(f"cp{i}_conv_w", [ADIM, ADIM, 3, 3, 3], wdt, kind="ExternalInput").ap() for i in (1, 2)]
    cp_up_w = [nc.dram_tensor(f"cp{i}_up_w", [ADIM, C], wdt, kind="ExternalInput").ap() for i in (1, 2)]

    def opt_vec(name, length):
        if not nz.get(name, False):
            return None
        return nc.dram_tensor(name, [length, 1], f32, kind="ExternalInput").ap()

    proj_b = opt_vec("proj_b", C)
    ff_b1 = opt_vec("ff_b1", MLP)
    ff_b2 = opt_vec("ff_b2", C)
    cp_down_b = [opt_vec(f"cp{i}_down_b", ADIM) for i in (1, 2)]
    cp_conv_b = [opt_vec(f"cp{i}_conv_b", ADIM) for i in (1, 2)]
    cp_up_b = [opt_vec(f"cp{i}_up_b", C) for i in (1, 2)]
    ln_g = [opt_vec("ln1_g", C), opt_vec("ln2_g", C)]
    ln_b = [opt_vec("ln1_b", C), opt_vec("ln2_b", C)]

    out = nc.dram_tensor("out", [C, TOK], f32, kind="ExternalOutput").ap()

    with tile.TileContext(nc) as tc:
        _build_tc(nc, tc, dict(
            xin=xin, pos=pos, qkv_w=qkv_w, proj_w=proj_w, ff_w1=ff_w1,
            ff_w2=ff_w2, cp_down_w=cp_down_w, cp_conv_w=cp_conv_w,
            cp_up_w=cp_up_w, proj_b=proj_b, ff_b1=ff_b1, ff_b2=ff_b2,
            cp_down_b=cp_down_b, cp_conv_b=cp_conv_b, cp_up_b=cp_up_b,
            ln_g=ln_g, ln_b=ln_b, out=out), sim_gelu=sim_gelu,
            gemm_bf16=gemm_bf16)
    return nc


def _build_tc(nc, tc, t, sim_gelu=False, gemm_bf16=True):
    gdt = bf16 if gemm_bf16 else f32r
    wdma = lambda eng_out, dram: nc.sync.dma_start(eng_out, dram)
    from contextlib import ExitStack

    bslc = lambda b: slice(b * N, (b + 1) * N)

    with ExitStack() as top:
        const = top.enter_context(tc.tile_pool(name="const", bufs=1))
        ones_f32 = const.tile([P, P], f32, name="ones_f32")
        nc.vector.memset(ones_f32[:], 1.0)
        # bf16 ones scaled by 1/C: LN stat matmuls produce means directly
        oinv_f32 = const.tile([P, P], f32, name="oinv_f32")
        nc.vector.memset(oinv_f32[:], 1.0 / C)
        ones_inv = const.tile([P, P], bf16, name="ones_inv")
        nc.vector.tensor_copy(ones_inv[:], oinv_f32[:])
        zeros_f32 = const.tile([P, 800], f32, name="zeros_f32")
        nc.vector.memset(zeros_f32[:], 0.0)
        eps_t = const.tile([P, 1], f32, name="eps_t")
        nc.vector.memset(eps_t[:], EPS)

        bias_pool = top.enter_context(tc.tile_pool(name="biases", bufs=1))
        projb_sb = _bias_tiles(nc, bias_pool, t["proj_b"], CT, "projb") if t["proj_b"] is not None else None
        ffb1_sb = _bias_tiles(nc, bias_pool, t["ff_b1"], MT1, "ffb1") if t["ff_b1"] is not None else None
        ffb2_sb = _bias_tiles(nc, bias_pool, t["ff_b2"], CT, "ffb2") if t["ff_b2"] is not None else None
        lng_sb = [None, None]
        lnb_sb = [None, None]
        for i in range(2):
            if t["ln_g"][i] is not None:
                lng_sb[i] = _bias_tiles(nc, bias_pool, t["ln_g"][i], CT, f"lng{i}")
            if t["ln_b"][i] is not None:
                lnb_sb[i] = _bias_tiles(nc, bias_pool, t["ln_b"][i], CT, f"lnb{i}")
        cp_db = [None, None]
        cp_cb = [None, None]
        cp_upb = [None, None]
        for i in range(2):
            if t["cp_down_b"][i] is not None:
                db = bias_pool.tile([96, 1], f32, name=f"db_{i}")
                nc.vector.tensor_copy(db[:], zeros_f32[0:96, 0:1])
                for dxi in range(3):
                    nc.sync.dma_start(db[32 * dxi:32 * dxi + ADIM, :], t["cp_down_b"][i][:])
                cp_db[i] = db
            if t["cp_conv_b"][i] is not None:
                cb = bias_pool.tile([ADIM, 1], f32, name=f"cb_{i}")
                nc.sync.dma_start(cb[:], t["cp_conv_b"][i][:])
                cp_cb[i] = cb
            if t["cp_up_b"][i] is not None:
                cp_upb[i] = _bias_tiles(nc, bias_pool, t["cp_up_b"][i], CT, f"upb{i}")

        # convpass weights (small, persistent). The 3 x-shifts of the conv
        # live at partition blocks 32*dxi (8 rows each); unused rows are
        # zeroed so they contribute nothing to the contraction.
        cpw_pool = top.enter_context(tc.tile_pool(name="cpw", bufs=1))
        dw96_sb = [[], []]
        w96_sb = [None, None]
        upw_sb = [None, None]
        for i in range(2):
            for ct in range(CT):
                dw96_sb[i].append(cpw_pool.tile([P, 96], gdt, name=f"dw96_{i}_{ct}"))
            w96_sb[i] = cpw_pool.tile([96, 9, ADIM], gdt, name=f"w96_{i}")
            upw_sb[i] = cpw_pool.tile([ADIM, C], gdt, name=f"upw_{i}")

        def load_cpw():
            # emitted after phase 0 so these small strided DMAs never sit
            # in the sync queue ahead of the x/pos input stream
            for i in range(2):
                for ct in range(CT):
                    dw96 = dw96_sb[i][ct]
                    nc.vector.tensor_copy(dw96[:], zeros_f32[:, 0:96])
                    for dxi in range(3):
                        wdma(dw96[:, 32 * dxi:32 * dxi + ADIM],
                             t["cp_down_w"][i][ct * P:(ct + 1) * P, :])
                w96 = w96_sb[i]
                nc.vector.tensor_copy(w96[:].rearrange("p a b -> p (a b)"), zeros_f32[0:96, 0:72])
                for dxi in range(3):
                    nc.gpsimd.dma_start(
                        w96[32 * dxi:32 * dxi + ADIM, :, :],
                        t["cp_conv_w"][i][:, :, :, :, dxi].rearrange("o i dz dy -> i (dz dy) o"))
                wdma(upw_sb[i][:], t["cp_up_w"][i][:])

        # persistent activations; xT doubles as the residual carrier
        pool_xT = top.enter_context(tc.tile_pool(name="xT", bufs=1))
        xT = [pool_xT.tile([P, TOK], f32r, name=f"xT{ct}") for ct in range(CT)]
        pool_hT = top.enter_context(tc.tile_pool(name="hT", bufs=1))
        hT = [pool_hT.tile([P, TOK], gdt, name=f"hT{ct}") for ct in range(CT)]
        x1T = xT

        # ---- attention weights: loaded after batch 0 of x/pos (below) ----
        att_es = ExitStack()
        wpool = att_es.enter_context(tc.tile_pool(name="attw", bufs=1))
        qkvw_sb = [wpool.tile([P, 3 * C], gdt, name=f"qkvw{ct}") for ct in range(CT)]
        projw_sb = [wpool.tile([P, C], gdt, name=f"projw{ct}") for ct in range(CT)]

        # ---- LayerNorm: stats via bf16 ones-matmuls (1/C folded in),
        #      Rsqrt on ScalarE; the normalize itself reads full-precision x
        def layer_norm_one(src, dst, which, b, lnps, lnsb, psname="ln_s"):
            xb = []
            sq = []
            for ct in range(CT):
                xb_t = lnsb.tile([P, N], bf16, name="ln_xb", bufs=4)
                nc.vector.tensor_copy(xb_t[:], src[ct][:, bslc(b)])
                xb.append(xb_t)
                sq_t = lnsb.tile([P, N], bf16, name="ln_sq", bufs=4)
                nc.scalar.activation(sq_t[:], src[ct][:, bslc(b)], AF.Square)
                sq.append(sq_t)
            m = lnps.tile([P, N], f32, name=psname)
            for ct in range(CT):
                nc.tensor.matmul(m[:], ones_inv[:], xb[ct][:],
                                 start=(ct == 0), stop=(ct == CT - 1))
            e2 = lnps.tile([P, N], f32, name=psname)
            for ct in range(CT):
                nc.tensor.matmul(e2[:], ones_inv[:], sq[ct][:],
                                 start=(ct == 0), stop=(ct == CT - 1))
            # stats on the [1,N] row (rows are identical), then broadcast.
            # rstd = exp(-0.5*ln(var+eps)): InstReciprocal costs ~8 cyc/elem
            # on DVE regardless of partition count, so use ScalarE LUTs.
            msq1 = lnsb.tile([1, N], f32, name="ln_msq1", bufs=1)
            nc.scalar.activation(msq1[:], m[0:1, :], AF.Square)
            var1 = lnsb.tile([1, N], f32, name="ln_var1", bufs=1)
            nc.vector.tensor_sub(var1[:], e2[0:1, :], msq1[:])
            lnv1 = lnsb.tile([1, N], f32, name="ln_lnv1", bufs=1)
            nc.scalar.activation(lnv1[:], var1[:], AF.Ln, bias=eps_t[0:1, :])
            rstd1 = lnsb.tile([1, N], f32, name="ln_rstd1", bufs=1)
            nc.scalar.activation(rstd1[:], lnv1[:], AF.Exp, scale=-0.5)
            mr1 = lnsb.tile([1, N], f32, name="ln_mr1", bufs=1)
            nc.vector.tensor_mul(mr1[:], m[0:1, :], rstd1[:])
            rstd = lnsb.tile([P, N], f32, name="ln_rstd", bufs=2)
            nc.gpsimd.partition_broadcast(rstd[:], rstd1[:])
            mr = lnsb.tile([P, N], f32, name="ln_mr", bufs=2)
            nc.gpsimd.partition_broadcast(mr[:], mr1[:])
            for ct in range(CT):
                tmp = lnsb.tile([P, N], f32, name="ln_tmp", bufs=2)
                nc.vector.tensor_mul(tmp[:], src[ct][:, bslc(b)], rstd[:])
                nc.vector.tensor_sub(dst[ct][:, bslc(b)], tmp[:], mr[:])
                if lng_sb[which] is not None or lnb_sb[which] is not None:
                    g = lng_sb[which][ct][:] if lng_sb[which] is not None else 1.0
                    bb = lnb_sb[which][ct][:] if lnb_sb[which] is not None else 0.0
                    nc.vector.tensor_scalar(dst[ct][:, bslc(b)], dst[ct][:, bslc(b)],
                                            g, bb, op0=ALU.mult, op1=ALU.add)

        def layer_norm(src, dst, which, es):
            lnps = es.enter_context(tc.tile_pool(name=f"ln{which}ps", bufs=2, space="PSUM"))
            lnsb = es.enter_context(tc.tile_pool(name=f"ln{which}sb", bufs=3))
            for b in range(BPC):
                layer_norm_one(src, dst, which, b, lnps, lnsb)

        # ---- Convpass, batch-staged so the PE never waits on the
        #      qgelu/im2col chain of the batch it is about to convolve ----
        def convpass_all(i, src, fold, es, after_batch=None):
            csb = es.enter_context(tc.tile_pool(name=f"cp{i}sb", bufs=1))
            dnps = es.enter_context(tc.tile_pool(name=f"cp{i}dn", bufs=4, space="PSUM"))
            cvps = es.enter_context(tc.tile_pool(name=f"cp{i}cv", bufs=2, space="PSUM"))
            upps = es.enter_context(tc.tile_pool(name=f"cp{i}up", bufs=2, space="PSUM"))
            ims, pts = [], []
            for b in range(BPC):
                d_ps = dnps.tile([96, N], f32, name="cp_dps")
                for ct in range(CT):
                    nc.tensor.matmul(d_ps[:], dw96_sb[i][ct][:], src[ct][:, bslc(b)],
                                     start=(ct == 0), stop=(ct == CT - 1))
                if cp_db[i] is not None:
                    dz = csb.tile([96, N], f32, name="cp_dz", bufs=4)
                    nc.vector.tensor_scalar_add(dz[:], d_ps[:], cp_db[i][:])
                    d_in = dz
                else:
                    d_in = d_ps
                sg = csb.tile([96, N], f32, name="cp_sg", bufs=4)
                nc.scalar.activation(sg[:], d_in[:], AF.Sigmoid, scale=QSCALE)
                d96 = csb.tile([96, N], gdt, name="cp_d96", bufs=4)
                nc.vector.tensor_mul(d96[:], d_in[:], sg[:])
                # fresh per-batch im2col buffer; zero it, then write the
                # interior. Block dxi holds in[..., x + dxi - 1].
                im96 = csb.tile([96, 10, 10, 8], gdt, name="cp_im96", bufs=4)
                nc.vector.tensor_copy(im96[:].rearrange("p a b c -> p (a b c)"),
                                      zeros_f32[0:96, 0:800])
                dv = d96[:].rearrange("p (z y x) -> p z y x", z=8, y=8)
                nc.vector.tensor_copy(im96[0:8, 1:9, 1:9, 1:8], dv[0:8, :, :, 0:7])
                nc.vector.tensor_copy(im96[32:40, 1:9, 1:9, 0:8], dv[32:40, :, :, 0:8])
                nc.vector.tensor_copy(im96[64:72, 1:9, 1:9, 0:7], dv[64:72, :, :, 1:8])
                ims.append(im96)
            for b in range(BPC):
                cv_ps = cvps.tile([ADIM, N], f32, name="cp_cvps")
                cv_view = cv_ps[:].rearrange("p (z y x) -> p z y x", z=8, y=8)
                for tap in range(9):
                    dzz, dyy = tap // 3, tap % 3
                    nc.tensor.matmul(cv_view, w96_sb[i][:, tap, :],
                                     ims[b][0:96, dzz:dzz + 8, dyy:dyy + 8, 0:8],
                                     start=(tap == 0), stop=(tap == 8))
                if cp_cb[i] is not None:
                    cz = csb.tile([ADIM, N], f32, name="cp_cz", bufs=4)
                    nc.vector.tensor_scalar_add(cz[:], cv_ps[:], cp_cb[i][:])
                    c_in = cz
                else:
                    c_in = cv_ps
                sg2 = csb.tile([ADIM, N], f32, name="cp_sg2", bufs=4)
                nc.scalar.activation(sg2[:], c_in[:], AF.Sigmoid, scale=QSCALE)
                pt = csb.tile([ADIM, N], gdt, name="cp_pt", bufs=4)
                nc.vector.tensor_mul(pt[:], c_in[:], sg2[:])
                pts.append(pt)
            for b in range(BPC):
                for ct in range(CT):
                    up_ps = upps.tile([P, N], f32, name="cp_upps")
                    nc.tensor.matmul(up_ps[:], upw_sb[i][:, ct * P:(ct + 1) * P],
                                     pts[b][:], start=True, stop=True)
                    fold(b, ct, up_ps)
                if after_batch is not None:
                    after_batch(b, upps)

        # ---- Phase 2: attention (+ proj, x1 = x + proj in place).
        #      QKV of batch b+1 is emitted before proj of batch b. ----
        with ExitStack() as esw:
            qk_pool = esw.enter_context(tc.tile_pool(name="qksb", bufs=2))
            v_pool = esw.enter_context(tc.tile_pool(name="vsb", bufs=1))
            e_pool = esw.enter_context(tc.tile_pool(name="esb", bufs=1))
            a_pool = esw.enter_context(tc.tile_pool(name="acsb", bufs=1))
            n_pool = esw.enter_context(tc.tile_pool(name="nsb", bufs=2))

            qkps = esw.enter_context(tc.tile_pool(name="qkps", bufs=2, space="PSUM"))
            scps = esw.enter_context(tc.tile_pool(name="scps", bufs=2, space="PSUM"))
            avps = esw.enter_context(tc.tile_pool(name="avps", bufs=4, space="PSUM"))
            p0sb = esw.enter_context(tc.tile_pool(name="p0", bufs=6))
            ln1sb = esw.enter_context(tc.tile_pool(name="ln0sb", bufs=3))

            def phase0_ln1(b):
                # load+add batch b, then LN1(b); stats matmuls share the
                # qk_ps PSUM ring to stay within the 8-bank budget
                for ct in range(CT):
                    xt = p0sb.tile([P, N], f32, name="xt_in")
                    pt = p0sb.tile([P, N], f32, name="pt_in")
                    nc.sync.dma_start(xt[:], t["xin"][ct * P:(ct + 1) * P, bslc(b)])
                    nc.sync.dma_start(pt[:], t["pos"][ct * P:(ct + 1) * P, bslc(b)])
                    nc.vector.tensor_add(xT[ct][:, bslc(b)], xt[:], pt[:])
                if b == 0:
                    for ct in range(CT):
                        wdma(qkvw_sb[ct][:], t["qkv_w"][ct * P:(ct + 1) * P, :])
                    for ct in range(CT):
                        wdma(projw_sb[ct][:], t["proj_w"][ct * P:(ct + 1) * P, :])
                layer_norm_one(xT, hT, 0, b, qkps, ln1sb, psname="qk_ps")

            def emit_qkv(b):
                qk_sb = []
                v_sb = []
                for mt in range(8):
                    qk_ps = qkps.tile([P, N], f32, name="qk_ps")
                    for ct in range(CT):
                        nc.tensor.matmul(qk_ps[:], qkvw_sb[ct][:, mt * P:(mt + 1) * P],
                                         hT[ct][:, bslc(b)],
                                         start=(ct == 0), stop=(ct == CT - 1))
                    if mt < 4:
                        # Q side: per-head tiles, other head's rows zeroed —
                        # the zeros make the full-width K tiles contract
                        # correctly at K=128 (partial-K matmuls lose FWL)
                        for hh in range(2):
                            qp = qk_pool.tile([P, N], gdt, name=f"q{mt}_{hh}")
                            if b < 2:  # bufs=2 slots keep their zero half
                                nc.vector.tensor_copy(
                                    qp[DH - hh * DH:P - hh * DH, :],
                                    zeros_f32[0:DH, 0:N])
                            nc.vector.tensor_copy(
                                qp[hh * DH:(hh + 1) * DH, :],
                                qk_ps[hh * DH:(hh + 1) * DH, :])
                            qk_sb.append((mt, hh, qp))
                    else:
                        # K side: one full-width copy per m-tile
                        qk_t = qk_pool.tile([P, N], gdt, name=f"k{mt}")
                        nc.vector.tensor_copy(qk_t[:], qk_ps[:])
                        qk_sb.append(qk_t)
                for s in range(NT):
                    v_ps = qkps.tile([P, C], f32, name="qk_ps")
                    for ct in range(CT):
                        nc.tensor.matmul(v_ps[:], hT[ct][:, b * N + s * P: b * N + (s + 1) * P],
                                         qkvw_sb[ct][:, 2 * C:3 * C],
                                         start=(ct == 0), stop=(ct == CT - 1))
                    # per-head stride 128 (full lhsT width => FWL); col DH
                    # holds the softmax-denominator ones, cols DH+1.. zeros
                    v_t = v_pool.tile([P, H * P], gdt, name=f"vt{s}")
                    vv = v_t[:].rearrange("p (h e) -> p h e", h=H)
                    nc.vector.tensor_copy(
                        vv[:, :, 0:DH],
                        v_ps[:].rearrange("p (h d) -> p h d", h=H))
                    nc.vector.tensor_copy(
                        vv[:, :, DH:DH + 1],
                        ones_f32[:, 0:H].rearrange("p (h o) -> p h o", o=1))
                    if b == 0:  # bufs=1 slot keeps its zero pad after first init
                        nc.vector.tensor_copy(
                            vv[:, :, DH + 1:P],
                            zeros_f32[:, 0:H * (P - DH - 1)].rearrange("p (h o) -> p h o", h=H))
                    v_sb.append(v_t)
                return qk_sb, v_sb

            def emit_scores(qk_sb, hp):
                e_sb = {}
                for hh in range(2):
                    for kt in range(NT):
                        sc_ps = scps.tile([P, N], f32, name="sc_ps", bufs=2)
                        nc.tensor.matmul(sc_ps[:],
                                         qk_sb[8 + hp][:, kt * P:(kt + 1) * P],
                                         qk_sb[2 * hp + hh][2][:],
                                         start=True, stop=True)
                        e_t = e_pool.tile([P, N], gdt, name=f"e{hp % 2}_{hh}_{kt}")
                        nc.scalar.activation(e_t[:], sc_ps[:], AF.Exp, scale=SCALE)
                        e_sb[(hh, kt)] = e_t
                return e_sb

            def emit_avmm(v_sb, den4, hp, e_sb, dk):
                # A@V matmuls for one pair; denominator rows land 32 apart
                # in the group's shared den4 tile
                avs = []
                for hh in range(2):
                    h = 2 * hp + hh
                    av_ps = avps.tile([P, N], f32, name="av_ps")
                    for kt in range(NT):
                        nc.tensor.matmul(av_ps[:],
                                         v_sb[kt][:, h * P:(h + 1) * P],
                                         e_sb[(hh, kt)][:],
                                         start=(kt == 0), stop=(kt == NT - 1))
                    avs.append(av_ps)
                    k = dk + hh
                    nc.vector.tensor_copy(den4[32 * k:32 * k + 1, :],
                                          av_ps[DH:DH + 1, :])
                return avs

            def emit_norm(ac_sb, g, den4, avs):
                # one InstReciprocal per 4 heads (cost ~8 cyc per free-size
                # element regardless of the partition count)
                rcp4 = n_pool.tile([97, N], f32, name="rcp4")
                nc.vector.reciprocal(rcp4[:], den4[:])
                for k in range(4):
                    h = 4 * g + k
                    if k == 0:
                        rsrc = rcp4[0:1, :]
                    else:
                        r1 = n_pool.tile([1, N], f32, name=f"r1_{k}")
                        nc.vector.tensor_copy(r1[:], rcp4[32 * k:32 * k + 1, :])
                        rsrc = r1[:]
                    rn = n_pool.tile([DH, N], f32, name="rn")
                    nc.gpsimd.partition_broadcast(rn[:], rsrc)
                    orow = (h % 2) * DH
                    nc.vector.tensor_tensor(
                        ac_sb[h // 2][orow:orow + DH, :],
                        avs[k][0:DH, :], rn[:], op=ALU.mult)

            def emit_proj(b, ac_sb):
                for ct in range(CT):
                    pr_ps = avps.tile([P, N], f32, name="av_ps")
                    for kt in range(CT):
                        nc.tensor.matmul(pr_ps[:], projw_sb[kt][:, ct * P:(ct + 1) * P],
                                         ac_sb[kt][:], start=(kt == 0), stop=(kt == CT - 1))
                    if projb_sb is not None:
                        prb = n_pool.tile([P, N], f32, name="prb")
                        nc.vector.tensor_scalar_add(prb[:], pr_ps[:], projb_sb[ct][:])
                        nc.vector.tensor_add(x1T[ct][:, bslc(b)], xT[ct][:, bslc(b)], prb[:])
                    else:
                        nc.vector.tensor_add(x1T[ct][:, bslc(b)], xT[ct][:, bslc(b)], pr_ps[:])

            phase0_ln1(0)
            qkv_cur = emit_qkv(0)
            for b in range(BPC):
                qk_sb, v_sb = qkv_cur
                ac_sb = [a_pool.tile([P, N], gdt, name=f"ac{ct}") for ct in range(CT)]
                # software-pipelined by pair-group: scores/exp of group 1
                # are emitted before the A@V of group 0, so the in-order PE
                # always has score matmuls to run while ScalarE exps.
                e0 = emit_scores(qk_sb, 0)
                e1 = emit_scores(qk_sb, 1)
                den_a = n_pool.tile([97, N], f32, name="den_a")
                avs_a = emit_avmm(v_sb, den_a, 0, e0, 0)
                avs_a += emit_avmm(v_sb, den_a, 1, e1, 2)
                if b + 1 < BPC:
                    phase0_ln1(b + 1)
                e2 = emit_scores(qk_sb, 2)
                e3 = emit_scores(qk_sb, 3)
                emit_norm(ac_sb, 0, den_a, avs_a)
                den_b = n_pool.tile([97, N], f32, name="den_b")
                avs_b = emit_avmm(v_sb, den_b, 2, e2, 0)
                avs_b += emit_avmm(v_sb, den_b, 3, e3, 2)
                if b + 1 < BPC:
                    qkv_cur = emit_qkv(b + 1)
                emit_norm(ac_sb, 1, den_b, avs_b)
                emit_proj(b, ac_sb)
        att_es.close()
        load_cpw()

        # ---- FFN weights: resident, loaded during the convpass1 window ----
        ffw_es = ExitStack()
        ffw_pool = ffw_es.enter_context(tc.tile_pool(name="ffw", bufs=1))
        w1_res = []
        w2_res = []
        for g in range(W1G):
            w1t = ffw_pool.tile([P, 2, 2, 8, P], f8, name=f"w1_{g}")
            wdma(w1t[:].rearrange("p a b c d -> p (a b c d)"), t["ff_w1"][g])
            w1_res.append(w1t)
        for g in range(W1G):
            w2t = ffw_pool.tile([P, 4, 2, C], f8, name=f"w2_{g}")
            wdma(w2t[:].rearrange("p a b c -> p (a b c)"), t["ff_w2"][g])
            w2_res.append(w2t)

        def w1_ap(mt, ctp):
            # [K=128, 2, M=128] DoubleRow stationary operand
            return w1_res[mt // 8][:, ctp, :, mt % 8, :]

        def w2_ap(mtp, ct):
            return w2_res[(2 * mtp) // 8][:, mtp % 4, :, ct * P:(ct + 1) * P]

        # LN2 output aliases hT; h8 (fp8 pair copy for the FFN GEMMs)
        # is cast at FFN phase start
        h2T = hT
        h8_pool = ffw_es.enter_context(tc.tile_pool(name="h8", bufs=1))
        h8 = [h8_pool.tile([P, 2, TOK], f8, name=f"h8_{cp}") for cp in range(2)]

        # ---- Phase 2b: convpass1, folded into x1 ----
        with ExitStack() as escp1:
            def fold1(b, ct, up_ps):
                if cp_upb[0] is not None:
                    ub = escp1_sb.tile([P, N], f32, name="upb_t", bufs=2)
                    nc.vector.tensor_scalar_add(ub[:], up_ps[:], cp_upb[0][ct][:])
                    nc.vector.tensor_add(x1T[ct][:, bslc(b)], x1T[ct][:, bslc(b)], ub[:])
                else:
                    nc.vector.tensor_add(x1T[ct][:, bslc(b)], x1T[ct][:, bslc(b)], up_ps[:])
            escp1_sb = escp1.enter_context(tc.tile_pool(name="cp1fold", bufs=1))
            convpass_all(0, hT, fold1, escp1)

        # ---- Phase 3: LN2 (h2 overwrites hT; convpass1 is done with it) ----
        with ExitStack() as es2:
            lnps2 = es2.enter_context(tc.tile_pool(name="ln2ps", bufs=2, space="PSUM"))
            lnsb2 = es2.enter_context(tc.tile_pool(name="ln2sb", bufs=3))
            for b in range(BPC):
                layer_norm_one(x1T, h2T, 1, b, lnps2, lnsb2, psname="ln2_ps")

        # ---- Phase 4: convpass2, folded into x1T ----
        with ExitStack() as escp2:
            def fold2(b, ct, up_ps):
                if cp_upb[1] is not None:
                    ub = escp2_sb.tile([P, N], f32, name="upb2_t", bufs=2)
                    nc.vector.tensor_scalar_add(ub[:], up_ps[:], cp_upb[1][ct][:])
                    nc.vector.tensor_add(x1T[ct][:, bslc(b)], x1T[ct][:, bslc(b)], ub[:])
                else:
                    nc.vector.tensor_add(x1T[ct][:, bslc(b)], x1T[ct][:, bslc(b)], up_ps[:])
            escp2_sb = escp2.enter_context(tc.tile_pool(name="cp2fold", bufs=1))
            convpass_all(1, h2T, fold2, escp2)

        # ---- Phase 5: fused FFN per batch + residual + store ----
        # f2 accumulates in PSUM across all 32 hidden m-tiles; f1 of tile m+1
        # is emitted before f2 of tile m so the PE never waits on the gelu.
        with tc.tile_pool(name="gmsb", bufs=3) as gmsb, \
             tc.tile_pool(name="outsb", bufs=4) as outsb, \
             tc.tile_pool(name="f1ps", bufs=3, space="PSUM") as f1ps, \
             tc.tile_pool(name="f2ps", bufs=1, space="PSUM") as f2ps:
            for b in range(BPC):
                for ct in range(CT):
                    nc.vector.tensor_copy(h8[ct // 2][:, ct % 2, bslc(b)],
                                          h2T[ct][:, bslc(b)])
            for b in range(BPC):
                f2acc = [f2ps.tile([P, N], f32, name=f"f2acc{ct}") for ct in range(CT)]

                def emit_f2(mtp, g8t, f2acc=f2acc):
                    for ct in range(CT):
                        nc.tensor.matmul(f2acc[ct][:], w2_ap(mtp, ct), g8t[:],
                                         start=(mtp == 0), stop=(mtp == MT1 // 2 - 1),
                                         perf_mode=DR)

                prev_g8 = None
                for mtp in range(MT1 // 2):
                    g8t = gmsb.tile([P, 2, N], f8, name="g8")
                    for jj in range(2):
                        mt = 2 * mtp + jj
                        f1_ps = f1ps.tile([P, N], f32, name="f1_ps")
                        for ctp in range(2):
                            nc.tensor.matmul(f1_ps[:], w1_ap(mt, ctp),
                                             h8[ctp][:, :, bslc(b)],
                                             start=(ctp == 0), stop=(ctp == 1),
                                             perf_mode=DR)
                        bias = ffb1_sb[mt][:] if ffb1_sb is not None else 0.0
                        if sim_gelu:
                            fsg = gmsb.tile([P, N], f32, name="fsg")
                            nc.scalar.activation(fsg[:], f1_ps[:], AF.Sigmoid,
                                                 scale=QSCALE, bias=bias)
                            nc.vector.tensor_mul(g8t[:, jj, :], f1_ps[:], fsg[:])
                        else:
                            nc.scalar.activation(g8t[:, jj, :], f1_ps[:], AF.Gelu,
                                                 bias=bias)
                    if prev_g8 is not None:
                        emit_f2(mtp - 1, prev_g8)
                    prev_g8 = g8t
                emit_f2(MT1 // 2 - 1, prev_g8)

                for ct in range(CT):
                    ofm = outsb.tile([P, N], f32, name="ofm")
                    if ffb2_sb is not None:
                        f2b = outsb.tile([P, N], f32, name="f2b")
                        nc.vector.tensor_scalar_add(f2b[:], f2acc[ct][:], ffb2_sb[ct][:])
                        nc.vector.tensor_add(ofm[:], x1T[ct][:, bslc(b)], f2b[:])
                    else:
                        nc.vector.tensor_add(ofm[:], x1T[ct][:, bslc(b)], f2acc[ct][:])
                    nc.sync.dma_start(
                        t["out"][ct * P:(ct + 1) * P, bslc(b)], ofm[:])
        ffw_es.close()


_CACHE = {}


def _get_compiled(nz_key, nz):
    if nz_key not in _CACHE:
        nc = bacc.Bacc("TRN2", target_bir_lowering=False, debug=False,
                       num_devices=NCORES)
        build(nc, nz)
        nc.compile()
        _CACHE[nz_key] = nc
    return _CACHE[nz_key]


def input_flags(inputs):
    nz = {}
    vec_names = ["proj_b", "ff_b1", "ff_b2", "cp1_down_b", "cp1_conv_b",
                 "cp1_up_b", "cp2_down_b", "cp2_conv_b", "cp2_up_b",
                 "ln1_b", "ln2_b"]
    for n in vec_names:
        nz[n] = bool(np.any(np.asarray(inputs[n]) != 0.0))
    nz["ln1_g"] = not bool(np.all(np.asarray(inputs["ln1_g"]) == 1.0))
    nz["ln2_g"] = not bool(np.all(np.asarray(inputs["ln2_g"]) == 1.0))
    return nz


def make_in_maps(inputs, nz):
    import ml_dtypes
    wnp = ml_dtypes.bfloat16 if GEMM_BF16 else np.float32
    x = np.asarray(inputs["x"], dtype=np.float32)
    pos = np.asarray(inputs["pos"], dtype=np.float32)
    common = {}
    for n in ["qkv_w", "proj_w"]:
        common[n] = np.ascontiguousarray(np.asarray(inputs[n], np.float32).astype(wnp))
    f8np = ml_dtypes.float8_e4m3fn
    # ff_w1 fp8 DoubleRow layout: [g, p, ctp, j, mtj, m], ct = 2*ctp + j
    w1 = np.asarray(inputs["ff_w1"], np.float32).reshape(2, 2, P, W1G, 8, P)
    common["ff_w1"] = np.ascontiguousarray(
        w1.transpose(3, 2, 0, 1, 4, 5).reshape(W1G, P, CT * 8 * P).astype(f8np))
    # ff_w2 fp8 DoubleRow layout: [g, p, mtpg, jj, m], mt = 2*mtp + jj
    w2 = np.asarray(inputs["ff_w2"], np.float32).reshape(W1G, 4, 2, P, C)
    common["ff_w2"] = np.ascontiguousarray(
        w2.transpose(0, 3, 1, 2, 4).reshape(W1G, P, 8 * C).astype(f8np))
    for i in (1, 2):
        for n in (f"cp{i}_down_w", f"cp{i}_conv_w", f"cp{i}_up_w"):
            common[n] = np.ascontiguousarray(np.asarray(inputs[n], np.float32).astype(wnp))
    for n, flag in nz.items():
        if flag:
            common[n] = np.ascontiguousarray(
                np.asarray(inputs[n], np.float32)).reshape(-1, 1)
    in_maps = []
    for c in range(NCORES):
        m = dict(common)
        m["x"] = np.ascontiguousarray(
            x[c * BPC:(c + 1) * BPC].transpose(2, 0, 1).reshape(C, TOK))
        m["pos"] = np.ascontiguousarray(
            pos[c * BPC:(c + 1) * BPC].transpose(2, 0, 1).reshape(C, TOK))
        in_maps.append(m)
    return in_maps


def kernel(**inputs):
    nz = input_flags(inputs)
    nz_key = tuple(sorted((k, v) for k, v in nz.items()))
    nc = _get_compiled(nz_key, nz)
    in_maps = make_in_maps(inputs, nz)
    res = run_bass_kernel_spmd(nc, in_maps, core_ids=list(range(NCORES)))
    out = np.concatenate(
        [res.results[c]["out"].reshape(C, BPC, N).transpose(1, 2, 0)
         for c in range(NCORES)], axis=0)
    return np.ascontiguousarray(out.astype(np.float32))


if __name__ == "__main__":
    # quick self-build check (no run)
    nc = bacc.Bacc("TRN2", target_bir_lowering=False, debug=False, num_devices=NCORES)
    build(nc, {})
    nc.compile()
    print("built + compiled OK; instructions:",
          sum(len(bb.instructions) for bb in nc.main_func.blocks))


# revision 35
# speedup vs baseline: 1.4074x; 1.0180x over previous
"""Trainium2 Bass kernel: ViT transformer block with Convpass adapters.

Problem nn_CTrans_42133629173960 (dense_transformer, compute-bound).

Sharding: pure data-parallel over batch — 8 NeuronCores x 4 batches each,
no collectives. On-chip layout is feature-major ([channel, token]); the host
pre-transposes x/pos (and un-transposes the output), so the device never
runs PE transposes and every GEMM contraction sits on the partition axis.

  - LayerNorm channel-reductions are ones-matmuls on the PE in bf16
    (1/C folded into the ones), Rsqrt on ScalarE.
  - Attention: scores are computed k-major (exp'd with ScalarE); the
    softmax denominator comes from a ones-column appended to V in the
    A@V matmul (PSUM row 64); the reciprocal runs on the [1,N] row
    before the partition-broadcast. V tiles are padded to a 128 stride
    per head so the A@V weight loads use fast-weight-load.
  - QKV of batch b+1 is emitted before proj of batch b so the PE stays
    busy through the softmax-normalize tail.
  - The 3x3x3 Convpass conv runs as 9 accumulated PE matmuls over a
    zero-padded (channel*dx, z, y, x) im2col buffer.
  - FFN weights live resident in SBUF (loaded once, during the convpass1
    window) in a host-prearranged contiguous tile layout.
  - Big GEMMs run in bf16 (weights pre-cast on the host); the residual
    carrier and LayerNorm statistics stay in f32r/fp32.

Self-contained: hardcodes shapes from the problem spec.
"""

import numpy as np

import concourse.bass as bass
import concourse.tile as tile
from concourse import bacc, mybir
from concourse.bass_utils import run_bass_kernel_spmd

f32 = mybir.dt.float32
f32r = mybir.dt.float32r
bf16 = mybir.dt.bfloat16
f8 = mybir.dt.float8e4
DR = mybir.MatmulPerfMode.DoubleRow
AF = mybir.ActivationFunctionType
ALU = mybir.AluOpType

B, N, C = 32, 512, 512
H, DH = 8, 64
ADIM = 8
MLP = 4096
EPS = 1e-5
SCALE = DH ** -0.5
NCORES = 8
BPC = B // NCORES          # 4 batches per core
TOK = BPC * N              # 2048 tokens per core
P = 128
CT = C // P                # 4 channel tiles
NT = N // P                # 4 token sub-tiles per batch
MT1 = MLP // P             # 32 tiles of the FFN hidden dim
W1G = 4                    # ff_w1 resident groups (8 m-tiles each)
QSCALE = 1.702             # quick-gelu sigmoid scale
GEMM_BF16 = True           # bf16 GEMM path (weights pre-cast on host)


def _bias_tiles(nc, pool, dram_ap, n_tiles, name):
    """Load a [n_tiles*128, 1] DRAM vector as per-partition scalar tiles."""
    tiles = []
    for t in range(n_tiles):
        bt = pool.tile([P, 1], f32, name=f"{name}{t}")
        nc.sync.dma_start(bt[:], dram_ap[t * P:(t + 1) * P, :])
        tiles.append(bt)
    return tiles


def build(nc, nz, sim_gelu=False, gemm_bf16=True):
    """Emit the per-core program. nz: dict of which biases are nonzero.
    sim_gelu: replace the FFN Gelu LUT with x*sigmoid(1.702x) (CoreSim has
    no Gelu implementation; only used by the simulation tests)."""
    wdt = bf16 if gemm_bf16 else f32r
    xin = nc.dram_tensor("x", [C, TOK], f32, kind="ExternalInput").ap()
    pos = nc.dram_tensor("pos", [C, TOK], f32, kind="ExternalInput").ap()
    qkv_w = nc.dram_tensor("qkv_w", [C, 3 * C], wdt, kind="ExternalInput").ap()
    proj_w = nc.dram_tensor("proj_w", [C, C], wdt, kind="ExternalInput").ap()
    # host-prearranged fp8 DoubleRow pair layouts (see make_in_maps)
    ff_w1 = nc.dram_tensor("ff_w1", [W1G, P, CT * 8 * P], f8, kind="ExternalInput").ap()
    ff_w2 = nc.dram_tensor("ff_w2", [W1G, P, 8 * C], f8, kind="ExternalInput").ap()
    cp_down_w = [nc.dram_tensor(f"cp{i}_down_w", [C, ADIM], wdt, kind="ExternalInput").ap() for i in (1, 2)]
    cp_conv_w = [nc.dram_tensor(f"cp{i}_conv_w", [ADIM, ADIM, 3, 3, 3], wdt, kind="ExternalInput").ap() for i in (1, 2)]
    cp_up_w = [nc.dram_tensor(f"cp{i}_up_w", [ADIM, C], wdt, kind="ExternalInput").ap() for i in (1, 2)]

    def opt_vec(name, length):
        if not nz.get(name, False):
            return None
        return nc.dram_tensor(name, [length, 1], f32, kind="ExternalInput").ap()

    proj_b = opt_vec("proj_b", C)
    ff_b1 = opt_vec("ff_b1", MLP)
    ff_b2 = opt_vec("ff_b2", C)
    cp_down_b = [opt_vec(f"cp{i}_down_b", ADIM) for i in (1, 2)]
    cp_conv_b = [opt_vec(f"cp{i}_conv_b", ADIM) for i in (1, 2)]
    cp_up_b = [opt_vec(f"cp{i}_up_b", C) for i in (1, 2)]
    ln_g = [opt_vec("ln1_g", C), opt_vec("ln2_g", C)]
    ln_b = [opt_vec("ln1_b", C), opt_vec("ln2_b", C)]

    out = nc.dram_tensor("out", [C, TOK], f32, kind="ExternalOutput").ap()

    with tile.TileContext(nc) as tc:
        _build_tc(nc, tc, dict(
            xin=xin, pos=pos, qkv_w=qkv_w, proj_w=proj_w, ff_w1=ff_w1,
            ff_w2=ff_w2, cp_down_w=cp_down_w, cp_conv_w=cp_conv_w,
            cp_up_w=cp_up_w, proj_b=proj_b, ff_b1=ff_b1, ff_b2=ff_b2,
            cp_down_b=cp_down_b, cp_conv_b=cp_conv_b, cp_up_b=cp_up_b,
            ln_g=ln_g, ln_b=ln_b, out=out), sim_gelu=sim_gelu,
            gemm_bf16=gemm_bf16)
    return nc


def _build_tc(nc, tc, t, sim_gelu=False, gemm_bf16=True):
    gdt = bf16 if gemm_bf16 else f32r
    wdma = lambda eng_out, dram: nc.sync.dma_start(eng_out, dram)
    from contextlib import ExitStack

    bslc = lambda b: slice(b * N, (b + 1) * N)

    with ExitStack() as top:
        const = top.enter_context(tc.tile_pool(name="const", bufs=1))
        ones_f32 = const.tile([P, P], f32, name="ones_f32")
        nc.vector.memset(ones_f32[:], 1.0)
        # bf16 ones scaled by 1/C: LN stat matmuls produce means directly
        oinv_f32 = const.tile([P, P], f32, name="oinv_f32")
        nc.vector.memset(oinv_f32[:], 1.0 / C)
        ones_inv = const.tile([P, P], bf16, name="ones_inv")
        nc.vector.tensor_copy(ones_inv[:], oinv_f32[:])
        zeros_f32 = const.tile([P, 800], f32, name="zeros_f32")
        nc.vector.memset(zeros_f32[:], 0.0)
        eps_t = const.tile([P, 1], f32, name="eps_t")
        nc.vector.memset(eps_t[:], EPS)

        bias_pool = top.enter_context(tc.tile_pool(name="biases", bufs=1))
        projb_sb = _bias_tiles(nc, bias_pool, t["proj_b"], CT, "projb") if t["proj_b"] is not None else None
        ffb1_sb = _bias_tiles(nc, bias_pool, t["ff_b1"], MT1, "ffb1") if t["ff_b1"] is not None else None
        ffb2_sb = _bias_tiles(nc, bias_pool, t["ff_b2"], CT, "ffb2") if t["ff_b2"] is not None else None
        lng_sb = [None, None]
        lnb_sb = [None, None]
        for i in range(2):
            if t["ln_g"][i] is not None:
                lng_sb[i] = _bias_tiles(nc, bias_pool, t["ln_g"][i], CT, f"lng{i}")
            if t["ln_b"][i] is not None:
                lnb_sb[i] = _bias_tiles(nc, bias_pool, t["ln_b"][i], CT, f"lnb{i}")
        cp_db = [None, None]
        cp_cb = [None, None]
        cp_upb = [None, None]
        for i in range(2):
            if t["cp_down_b"][i] is not None:
                db = bias_pool.tile([96, 1], f32, name=f"db_{i}")
                nc.vector.tensor_copy(db[:], zeros_f32[0:96, 0:1])
                for dxi in range(3):
                    nc.sync.dma_start(db[32 * dxi:32 * dxi + ADIM, :], t["cp_down_b"][i][:])
                cp_db[i] = db
            if t["cp_conv_b"][i] is not None:
                cb = bias_pool.tile([ADIM, 1], f32, name=f"cb_{i}")
                nc.sync.dma_start(cb[:], t["cp_conv_b"][i][:])
                cp_cb[i] = cb
            if t["cp_up_b"][i] is not None:
                cp_upb[i] = _bias_tiles(nc, bias_pool, t["cp_up_b"][i], CT, f"upb{i}")

        # convpass weights (small, persistent). The 3 x-shifts of the conv
        # live at partition blocks 32*dxi (8 rows each); unused rows are
        # zeroed so they contribute nothing to the contraction.
        cpw_pool = top.enter_context(tc.tile_pool(name="cpw", bufs=1))
        dw96_sb = [[], []]
        w96_sb = [None, None]
        upw_sb = [None, None]
        for i in range(2):
            for ct in range(CT):
                dw96_sb[i].append(cpw_pool.tile([P, 96], gdt, name=f"dw96_{i}_{ct}"))
            w96_sb[i] = cpw_pool.tile([96, 9, ADIM], gdt, name=f"w96_{i}")
            upw_sb[i] = cpw_pool.tile([ADIM, C], gdt, name=f"upw_{i}")

        def load_cpw():
            # emitted after phase 0 so these small strided DMAs never sit
            # in the sync queue ahead of the x/pos input stream
            for i in range(2):
                for ct in range(CT):
                    dw96 = dw96_sb[i][ct]
                    nc.vector.tensor_copy(dw96[:], zeros_f32[:, 0:96])
                    for dxi in range(3):
                        wdma(dw96[:, 32 * dxi:32 * dxi + ADIM],
                             t["cp_down_w"][i][ct * P:(ct + 1) * P, :])
                w96 = w96_sb[i]
                nc.vector.tensor_copy(w96[:].rearrange("p a b -> p (a b)"), zeros_f32[0:96, 0:72])
                for dxi in range(3):
                    nc.gpsimd.dma_start(
                        w96[32 * dxi:32 * dxi + ADIM, :, :],
                        t["cp_conv_w"][i][:, :, :, :, dxi].rearrange("o i dz dy -> i (dz dy) o"))
                wdma(upw_sb[i][:], t["cp_up_w"][i][:])

        # persistent activations; xT doubles as the residual carrier
        pool_xT = top.enter_context(tc.tile_pool(name="xT", bufs=1))
        xT = [pool_xT.tile([P, TOK], f32r, name=f"xT{ct}") for ct in range(CT)]
        pool_hT = top.enter_context(tc.tile_pool(name="hT", bufs=1))
        hT = [pool_hT.tile([P, TOK], gdt, name=f"hT{ct}") for ct in range(CT)]
        x1T = xT

        # ---- attention weights: loaded after batch 0 of x/pos (below) ----
        att_es = ExitStack()
        wpool = att_es.enter_context(tc.tile_pool(name="attw", bufs=1))
        qkvw_sb = [wpool.tile([P, 3 * C], gdt, name=f"qkvw{ct}") for ct in range(CT)]
        projw_sb = [wpool.tile([P, C], gdt, name=f"projw{ct}") for ct in range(CT)]

        # ---- LayerNorm: stats via bf16 ones-matmuls (1/C folded in),
        #      Rsqrt on ScalarE; the normalize itself reads full-precision x
        def layer_norm_one(src, dst, which, b, lnps, lnsb, psname="ln_s"):
            xb = []
            sq = []
            for ct in range(CT):
                xb_t = lnsb.tile([P, N], bf16, name="ln_xb", bufs=4)
                nc.vector.tensor_copy(xb_t[:], src[ct][:, bslc(b)])
                xb.append(xb_t)
                sq_t = lnsb.tile([P, N], bf16, name="ln_sq", bufs=4)
                nc.scalar.activation(sq_t[:], src[ct][:, bslc(b)], AF.Square)
                sq.append(sq_t)
            m = lnps.tile([P, N], f32, name=psname)
            for ct in range(CT):
                nc.tensor.matmul(m[:], ones_inv[:], xb[ct][:],
                                 start=(ct == 0), stop=(ct == CT - 1))
            e2 = lnps.tile([P, N], f32, name=psname)
            for ct in range(CT):
                nc.tensor.matmul(e2[:], ones_inv[:], sq[ct][:],
                                 start=(ct == 0), stop=(ct == CT - 1))
            # stats on the [1,N] row (rows are identical), then broadcast.
            # rstd = exp(-0.5*ln(var+eps)): InstReciprocal costs ~8 cyc/elem
            # on DVE regardless of partition count, so use ScalarE LUTs.
            msq1 = lnsb.tile([1, N], f32, name="ln_msq1", bufs=1)
            nc.scalar.activation(msq1[:], m[0:1, :], AF.Square)
            var1 = lnsb.tile([1, N], f32, name="ln_var1", bufs=1)
            nc.vector.tensor_sub(var1[:], e2[0:1, :], msq1[:])
            lnv1 = lnsb.tile([1, N], f32, name="ln_lnv1", bufs=1)
            nc.scalar.activation(lnv1[:], var1[:], AF.Ln, bias=eps_t[0:1, :])
            rstd1 = lnsb.tile([1, N], f32, name="ln_rstd1", bufs=1)
            nc.scalar.activation(rstd1[:], lnv1[:], AF.Exp, scale=-0.5)
            mr1 = lnsb.tile([1, N], f32, name="ln_mr1", bufs=1)
            nc.vector.tensor_mul(mr1[:], m[0:1, :], rstd1[:])
            rstd = lnsb.tile([P, N], f32, name="ln_rstd", bufs=2)
            nc.gpsimd.partition_broadcast(rstd[:], rstd1[:])
            mr = lnsb.tile([P, N], f32, name="ln_mr", bufs=2)
            nc.gpsimd.partition_broadcast(mr[:], mr1[:])
            for ct in range(CT):
                tmp = lnsb.tile([P, N], f32, name="ln_tmp", bufs=2)
                nc.vector.tensor_mul(tmp[:], src[ct][:, bslc(b)], rstd[:])
                nc.vector.tensor_sub(dst[ct][:, bslc(b)], tmp[:], mr[:])
                if lng_sb[which] is not None or lnb_sb[which] is not None:
                    g = lng_sb[which][ct][:] if lng_sb[which] is not None else 1.0
                    bb = lnb_sb[which][ct][:] if lnb_sb[which] is not None else 0.0
                    nc.vector.tensor_scalar(dst[ct][:, bslc(b)], dst[ct][:, bslc(b)],
                                            g, bb, op0=ALU.mult, op1=ALU.add)

        def layer_norm(src, dst, which, es):
            lnps = es.enter_context(tc.tile_pool(name=f"ln{which}ps", bufs=2, space="PSUM"))
            lnsb = es.enter_context(tc.tile_pool(name=f"ln{which}sb", bufs=3))
            for b in range(BPC):
                layer_norm_one(src, dst, which, b, lnps, lnsb)

        # ---- Convpass, batch-staged so the PE never waits on the
        #      qgelu/im2col chain of the batch it is about to convolve ----
        def convpass_all(i, src, fold, es, after_batch=None, pre_batch=None):
            csb = es.enter_context(tc.tile_pool(name=f"cp{i}sb", bufs=1))
            dnps = es.enter_context(tc.tile_pool(name=f"cp{i}dn", bufs=2, space="PSUM"))
            cvps = es.enter_context(tc.tile_pool(name=f"cp{i}cv", bufs=2, space="PSUM"))
            upps = es.enter_context(tc.tile_pool(name=f"cp{i}up", bufs=2, space="PSUM"))
            ims, pts = [], []
            for b in range(BPC):
                if pre_batch is not None:
                    pre_batch(b)
                d_ps = dnps.tile([96, N], f32, name="cp_dps")
                for ct in range(CT):
                    nc.tensor.matmul(d_ps[:], dw96_sb[i][ct][:], src[ct][:, bslc(b)],
                                     start=(ct == 0), stop=(ct == CT - 1))
                if cp_db[i] is not None:
                    dz = csb.tile([96, N], f32, name="cp_dz", bufs=4)
                    nc.vector.tensor_scalar_add(dz[:], d_ps[:], cp_db[i][:])
                    d_in = dz
                else:
                    d_in = d_ps
                sg = csb.tile([96, N], f32, name="cp_sg", bufs=4)
                nc.scalar.activation(sg[:], d_in[:], AF.Sigmoid, scale=QSCALE)
                d96 = csb.tile([96, N], gdt, name="cp_d96", bufs=4)
                nc.vector.tensor_mul(d96[:], d_in[:], sg[:])
                # fresh per-batch im2col buffer; zero it, then write the
                # interior. Block dxi holds in[..., x + dxi - 1].
                im96 = csb.tile([96, 10, 10, 8], gdt, name="cp_im96", bufs=4)
                nc.vector.tensor_copy(im96[:].rearrange("p a b c -> p (a b c)"),
                                      zeros_f32[0:96, 0:800])
                dv = d96[:].rearrange("p (z y x) -> p z y x", z=8, y=8)
                nc.vector.tensor_copy(im96[0:8, 1:9, 1:9, 1:8], dv[0:8, :, :, 0:7])
                nc.vector.tensor_copy(im96[32:40, 1:9, 1:9, 0:8], dv[32:40, :, :, 0:8])
                nc.vector.tensor_copy(im96[64:72, 1:9, 1:9, 0:7], dv[64:72, :, :, 1:8])
                ims.append(im96)
            for b in range(BPC):
                cv_ps = cvps.tile([ADIM, N], f32, name="cp_cvps")
                cv_view = cv_ps[:].rearrange("p (z y x) -> p z y x", z=8, y=8)
                for tap in range(9):
                    dzz, dyy = tap // 3, tap % 3
                    nc.tensor.matmul(cv_view, w96_sb[i][:, tap, :],
                                     ims[b][0:96, dzz:dzz + 8, dyy:dyy + 8, 0:8],
                                     start=(tap == 0), stop=(tap == 8))
                if cp_cb[i] is not None:
                    cz = csb.tile([ADIM, N], f32, name="cp_cz", bufs=4)
                    nc.vector.tensor_scalar_add(cz[:], cv_ps[:], cp_cb[i][:])
                    c_in = cz
                else:
                    c_in = cv_ps
                sg2 = csb.tile([ADIM, N], f32, name="cp_sg2", bufs=4)
                nc.scalar.activation(sg2[:], c_in[:], AF.Sigmoid, scale=QSCALE)
                pt = csb.tile([ADIM, N], gdt, name="cp_pt", bufs=4)
                nc.vector.tensor_mul(pt[:], c_in[:], sg2[:])
                pts.append(pt)
            for b in range(BPC):
                for ct in range(CT):
                    up_ps = upps.tile([P, N], f32, name="cp_upps")
                    nc.tensor.matmul(up_ps[:], upw_sb[i][:, ct * P:(ct + 1) * P],
                                     pts[b][:], start=True, stop=True)
                    fold(b, ct, up_ps)
                if after_batch is not None:
                    after_batch(b, upps)

        # ---- Phase 2: attention (+ proj, x1 = x + proj in place).
        #      QKV of batch b+1 is emitted before proj of batch b. ----
        with ExitStack() as esw:
            qk_pool = esw.enter_context(tc.tile_pool(name="qksb", bufs=2))
            v_pool = esw.enter_context(tc.tile_pool(name="vsb", bufs=1))
            e_pool = esw.enter_context(tc.tile_pool(name="esb", bufs=1))
            a_pool = esw.enter_context(tc.tile_pool(name="acsb", bufs=1))
            n_pool = esw.enter_context(tc.tile_pool(name="nsb", bufs=2))

            qkps = esw.enter_context(tc.tile_pool(name="qkps", bufs=2, space="PSUM"))
            scps = esw.enter_context(tc.tile_pool(name="scps", bufs=2, space="PSUM"))
            avps = esw.enter_context(tc.tile_pool(name="avps", bufs=4, space="PSUM"))
            p0sb = esw.enter_context(tc.tile_pool(name="p0", bufs=6))
            ln1sb = esw.enter_context(tc.tile_pool(name="ln0sb", bufs=3))

            def phase0_ln1(b):
                # load+add batch b, then LN1(b); stats matmuls share the
                # qk_ps PSUM ring to stay within the 8-bank budget
                for ct in range(CT):
                    xt = p0sb.tile([P, N], f32, name="xt_in")
                    pt = p0sb.tile([P, N], f32, name="pt_in")
                    nc.sync.dma_start(xt[:], t["xin"][ct * P:(ct + 1) * P, bslc(b)])
                    nc.sync.dma_start(pt[:], t["pos"][ct * P:(ct + 1) * P, bslc(b)])
                    nc.vector.tensor_add(xT[ct][:, bslc(b)], xt[:], pt[:])
                if b == 0:
                    for ct in range(CT):
                        wdma(qkvw_sb[ct][:], t["qkv_w"][ct * P:(ct + 1) * P, :])
                    for ct in range(CT):
                        wdma(projw_sb[ct][:], t["proj_w"][ct * P:(ct + 1) * P, :])
                layer_norm_one(xT, hT, 0, b, qkps, ln1sb, psname="qk_ps")

            def emit_qkv(b):
                qk_sb = []
                v_sb = []
                for mt in range(8):
                    qk_ps = qkps.tile([P, N], f32, name="qk_ps")
                    for ct in range(CT):
                        nc.tensor.matmul(qk_ps[:], qkvw_sb[ct][:, mt * P:(mt + 1) * P],
                                         hT[ct][:, bslc(b)],
                                         start=(ct == 0), stop=(ct == CT - 1))
                    if mt < 4:
                        # Q side: per-head tiles, other head's rows zeroed —
                        # the zeros make the full-width K tiles contract
                        # correctly at K=128 (partial-K matmuls lose FWL)
                        for hh in range(2):
                            qp = qk_pool.tile([P, N], gdt, name=f"q{mt}_{hh}")
                            if b < 2:  # bufs=2 slots keep their zero half
                                nc.vector.tensor_copy(
                                    qp[DH - hh * DH:P - hh * DH, :],
                                    zeros_f32[0:DH, 0:N])
                            nc.vector.tensor_copy(
                                qp[hh * DH:(hh + 1) * DH, :],
                                qk_ps[hh * DH:(hh + 1) * DH, :])
                            qk_sb.append((mt, hh, qp))
                    else:
                        # K side: one full-width copy per m-tile
                        qk_t = qk_pool.tile([P, N], gdt, name=f"k{mt}")
                        nc.vector.tensor_copy(qk_t[:], qk_ps[:])
                        qk_sb.append(qk_t)
                for s in range(NT):
                    v_ps = qkps.tile([P, C], f32, name="qk_ps")
                    for ct in range(CT):
                        nc.tensor.matmul(v_ps[:], hT[ct][:, b * N + s * P: b * N + (s + 1) * P],
                                         qkvw_sb[ct][:, 2 * C:3 * C],
                                         start=(ct == 0), stop=(ct == CT - 1))
                    # per-head stride 128 (full lhsT width => FWL); col DH
                    # holds the softmax-denominator ones, cols DH+1.. zeros
                    v_t = v_pool.tile([P, H * P], gdt, name=f"vt{s}")
                    vv = v_t[:].rearrange("p (h e) -> p h e", h=H)
                    nc.vector.tensor_copy(
                        vv[:, :, 0:DH],
                        v_ps[:].rearrange("p (h d) -> p h d", h=H))
                    nc.vector.tensor_copy(
                        vv[:, :, DH:DH + 1],
                        ones_f32[:, 0:H].rearrange("p (h o) -> p h o", o=1))
                    if b == 0:  # bufs=1 slot keeps its zero pad after first init
                        nc.vector.tensor_copy(
                            vv[:, :, DH + 1:P],
                            zeros_f32[:, 0:H * (P - DH - 1)].rearrange("p (h o) -> p h o", h=H))
                    v_sb.append(v_t)
                return qk_sb, v_sb

            def emit_scores(qk_sb, hp):
                e_sb = {}
                for hh in range(2):
                    for kt in range(NT):
                        sc_ps = scps.tile([P, N], f32, name="sc_ps", bufs=2)
                        nc.tensor.matmul(sc_ps[:],
                                         qk_sb[8 + hp][:, kt * P:(kt + 1) * P],
                                         qk_sb[2 * hp + hh][2][:],
                                         start=True, stop=True)
                        e_t = e_pool.tile([P, N], gdt, name=f"e{hp % 2}_{hh}_{kt}")
                        nc.scalar.activation(e_t[:], sc_ps[:], AF.Exp, scale=SCALE)
                        e_sb[(hh, kt)] = e_t
                return e_sb

            def emit_avmm(v_sb, den4, hp, e_sb, dk):
                # A@V matmuls for one pair; denominator rows land 32 apart
                # in the group's shared den4 tile
                avs = []
                for hh in range(2):
                    h = 2 * hp + hh
                    av_ps = avps.tile([P, N], f32, name="av_ps")
                    for kt in range(NT):
                        nc.tensor.matmul(av_ps[:],
                                         v_sb[kt][:, h * P:(h + 1) * P],
                                         e_sb[(hh, kt)][:],
                                         start=(kt == 0), stop=(kt == NT - 1))
                    avs.append(av_ps)
                    k = dk + hh
                    nc.vector.tensor_copy(den4[32 * k:32 * k + 1, :],
                                          av_ps[DH:DH + 1, :])
                return avs

            def emit_norm(ac_sb, g, den4, avs):
                # one InstReciprocal per 4 heads (cost ~8 cyc per free-size
                # element regardless of the partition count)
                rcp4 = n_pool.tile([97, N], f32, name="rcp4")
                nc.vector.reciprocal(rcp4[:], den4[:])
                for k in range(4):
                    h = 4 * g + k
                    if k == 0:
                        rsrc = rcp4[0:1, :]
                    else:
                        r1 = n_pool.tile([1, N], f32, name=f"r1_{k}")
                        nc.vector.tensor_copy(r1[:], rcp4[32 * k:32 * k + 1, :])
                        rsrc = r1[:]
                    rn = n_pool.tile([DH, N], f32, name="rn")
                    nc.gpsimd.partition_broadcast(rn[:], rsrc)
                    orow = (h % 2) * DH
                    nc.vector.tensor_tensor(
                        ac_sb[h // 2][orow:orow + DH, :],
                        avs[k][0:DH, :], rn[:], op=ALU.mult)

            def emit_proj(b, ac_sb):
                for ct in range(CT):
                    pr_ps = avps.tile([P, N], f32, name="av_ps")
                    for kt in range(CT):
                        nc.tensor.matmul(pr_ps[:], projw_sb[kt][:, ct * P:(ct + 1) * P],
                                         ac_sb[kt][:], start=(kt == 0), stop=(kt == CT - 1))
                    if projb_sb is not None:
                        prb = n_pool.tile([P, N], f32, name="prb")
                        nc.vector.tensor_scalar_add(prb[:], pr_ps[:], projb_sb[ct][:])
                        nc.vector.tensor_add(x1T[ct][:, bslc(b)], xT[ct][:, bslc(b)], prb[:])
                    else:
                        nc.vector.tensor_add(x1T[ct][:, bslc(b)], xT[ct][:, bslc(b)], pr_ps[:])

            phase0_ln1(0)
            qkv_cur = emit_qkv(0)
            for b in range(BPC):
                qk_sb, v_sb = qkv_cur
                ac_sb = [a_pool.tile([P, N], gdt, name=f"ac{ct}") for ct in range(CT)]
                # software-pipelined by pair-group: scores/exp of group 1
                # are emitted before the A@V of group 0, so the in-order PE
                # always has score matmuls to run while ScalarE exps.
                e0 = emit_scores(qk_sb, 0)
                e1 = emit_scores(qk_sb, 1)
                den_a = n_pool.tile([97, N], f32, name="den_a")
                avs_a = emit_avmm(v_sb, den_a, 0, e0, 0)
                avs_a += emit_avmm(v_sb, den_a, 1, e1, 2)
                if b + 1 < BPC:
                    phase0_ln1(b + 1)
                e2 = emit_scores(qk_sb, 2)
                e3 = emit_scores(qk_sb, 3)
                emit_norm(ac_sb, 0, den_a, avs_a)
                den_b = n_pool.tile([97, N], f32, name="den_b")
                avs_b = emit_avmm(v_sb, den_b, 2, e2, 0)
                avs_b += emit_avmm(v_sb, den_b, 3, e3, 2)
                if b + 1 < BPC:
                    qkv_cur = emit_qkv(b + 1)
                emit_norm(ac_sb, 1, den_b, avs_b)
                emit_proj(b, ac_sb)
        att_es.close()
        load_cpw()

        # ---- FFN weights: resident, loaded during the convpass1 window ----
        ffw_es = ExitStack()
        ffw_pool = ffw_es.enter_context(tc.tile_pool(name="ffw", bufs=1))
        w1_res = []
        w2_res = []
        for g in range(W1G):
            w1t = ffw_pool.tile([P, 2, 2, 8, P], f8, name=f"w1_{g}")
            wdma(w1t[:].rearrange("p a b c d -> p (a b c d)"), t["ff_w1"][g])
            w1_res.append(w1t)
        for g in range(W1G):
            w2t = ffw_pool.tile([P, 4, 2, C], f8, name=f"w2_{g}")
            wdma(w2t[:].rearrange("p a b c -> p (a b c)"), t["ff_w2"][g])
            w2_res.append(w2t)

        def w1_ap(mt, ctp):
            # [K=128, 2, M=128] DoubleRow stationary operand
            return w1_res[mt // 8][:, ctp, :, mt % 8, :]

        def w2_ap(mtp, ct):
            return w2_res[(2 * mtp) // 8][:, mtp % 4, :, ct * P:(ct + 1) * P]

        # LN2 output aliases hT; h8 (fp8 pair copy for the FFN GEMMs)
        # is cast at FFN phase start
        h2T = hT
        h8_pool = ffw_es.enter_context(tc.tile_pool(name="h8", bufs=1))
        h8 = [h8_pool.tile([P, 2, TOK], f8, name=f"h8_{cp}") for cp in range(2)]

        # ---- Phase 2b: convpass1, folded into x1 ----
        with ExitStack() as escp1:
            def fold1(b, ct, up_ps):
                if cp_upb[0] is not None:
                    ub = escp1_sb.tile([P, N], f32, name="upb_t", bufs=2)
                    nc.vector.tensor_scalar_add(ub[:], up_ps[:], cp_upb[0][ct][:])
                    nc.vector.tensor_add(x1T[ct][:, bslc(b)], x1T[ct][:, bslc(b)], ub[:])
                else:
                    nc.vector.tensor_add(x1T[ct][:, bslc(b)], x1T[ct][:, bslc(b)], up_ps[:])
            escp1_sb = escp1.enter_context(tc.tile_pool(name="cp1fold", bufs=1))
            convpass_all(0, hT, fold1, escp1)

        # ---- Phase 3: LN2 is fused into convpass2's first stage below,
        #      so conv2's down-matmuls keep the PE busy during LN2's DVE
        #      chain (LN2(b) runs right before conv2-down(b) reads h2T(b)) --

        # ---- Phase 4: convpass2, folded into x1T ----
        with ExitStack() as escp2:
            def fold2(b, ct, up_ps):
                if cp_upb[1] is not None:
                    ub = escp2_sb.tile([P, N], f32, name="upb2_t", bufs=2)
                    nc.vector.tensor_scalar_add(ub[:], up_ps[:], cp_upb[1][ct][:])
                    nc.vector.tensor_add(x1T[ct][:, bslc(b)], x1T[ct][:, bslc(b)], ub[:])
                else:
                    nc.vector.tensor_add(x1T[ct][:, bslc(b)], x1T[ct][:, bslc(b)], up_ps[:])
            escp2_sb = escp2.enter_context(tc.tile_pool(name="cp2fold", bufs=1))
            lnps2 = escp2.enter_context(tc.tile_pool(name="ln2ps", bufs=2, space="PSUM"))
            lnsb2 = escp2.enter_context(tc.tile_pool(name="ln2sb", bufs=3))

            def ln2_pre(b):
                layer_norm_one(x1T, h2T, 1, b, lnps2, lnsb2, psname="ln2_ps")
            convpass_all(1, h2T, fold2, escp2, pre_batch=ln2_pre)

        # ---- Phase 5: fused FFN per batch + residual + store ----
        # f2 accumulates in PSUM across all 32 hidden m-tiles; f1 of tile m+1
        # is emitted before f2 of tile m so the PE never waits on the gelu.
        with tc.tile_pool(name="gmsb", bufs=3) as gmsb, \
             tc.tile_pool(name="outsb", bufs=4) as outsb, \
             tc.tile_pool(name="f1ps", bufs=3, space="PSUM") as f1ps, \
             tc.tile_pool(name="f2ps", bufs=1, space="PSUM") as f2ps:
            for b in range(BPC):
                for ct in range(CT):
                    nc.vector.tensor_copy(h8[ct // 2][:, ct % 2, bslc(b)],
                                          h2T[ct][:, bslc(b)])
            for b in range(BPC):
                f2acc = [f2ps.tile([P, N], f32, name=f"f2acc{ct}") for ct in range(CT)]

                def emit_f2(mtp, g8t, f2acc=f2acc):
                    for ct in range(CT):
                        nc.tensor.matmul(f2acc[ct][:], w2_ap(mtp, ct), g8t[:],
                                         start=(mtp == 0), stop=(mtp == MT1 // 2 - 1),
                                         perf_mode=DR)

                prev_g8 = None
                for mtp in range(MT1 // 2):
                    g8t = gmsb.tile([P, 2, N], f8, name="g8")
                    for jj in range(2):
                        mt = 2 * mtp + jj
                        f1_ps = f1ps.tile([P, N], f32, name="f1_ps")
                        for ctp in range(2):
                            nc.tensor.matmul(f1_ps[:], w1_ap(mt, ctp),
                                             h8[ctp][:, :, bslc(b)],
                                             start=(ctp == 0), stop=(ctp == 1),
                                             perf_mode=DR)
                        bias = ffb1_sb[mt][:] if ffb1_sb is not None else 0.0
                        if sim_gelu:
                            fsg = gmsb.tile([P, N], f32, name="fsg")
                            nc.scalar.activation(fsg[:], f1_ps[:], AF.Sigmoid,
                                                 scale=QSCALE, bias=bias)
                            nc.vector.tensor_mul(g8t[:, jj, :], f1_ps[:], fsg[:])
                        else:
                            nc.scalar.activation(g8t[:, jj, :], f1_ps[:], AF.Gelu,
                                                 bias=bias)
                    if prev_g8 is not None:
                        emit_f2(mtp - 1, prev_g8)
                    prev_g8 = g8t
                emit_f2(MT1 // 2 - 1, prev_g8)

                for ct in range(CT):
                    ofm = outsb.tile([P, N], f32, name="ofm")
                    if ffb2_sb is not None:
                        f2b = outsb.tile([P, N], f32, name="f2b")
                        nc.vector.tensor_scalar_add(f2b[:], f2acc[ct][:], ffb2_sb[ct][:])
                        nc.vector.tensor_add(ofm[:], x1T[ct][:, bslc(b)], f2b[:])
                    else:
                        nc.vector.tensor_add(ofm[:], x1T[ct][:, bslc(b)], f2acc[ct][:])
                    nc.sync.dma_start(
                        t["out"][ct * P:(ct + 1) * P, bslc(b)], ofm[:])
        ffw_es.close()


_CACHE = {}


def _get_compiled(nz_key, nz):
    if nz_key not in _CACHE:
        nc = bacc.Bacc("TRN2", target_bir_lowering=False, debug=False,
                       num_devices=NCORES)
        build(nc, nz)
        nc.compile()
        _CACHE[nz_key] = nc
    return _CACHE[nz_key]


def input_flags(inputs):
    nz = {}
    vec_names = ["proj_b", "ff_b1", "ff_b2", "cp1_down_b", "cp1_conv_b",
                 "cp1_up_b", "cp2_down_b", "cp2_conv_b", "cp2_up_b",
                 "ln1_b", "ln2_b"]
    for n in vec_names:
        nz[n] = bool(np.any(np.asarray(inputs[n]) != 0.0))
    nz["ln1_g"] = not bool(np.all(np.asarray(inputs["ln1_g"]) == 1.0))
    nz["ln2_g"] = not bool(np.all(np.asarray(inputs["ln2_g"]) == 1.0))
    return nz


def make_in_maps(inputs, nz):
    import ml_dtypes
    wnp = ml_dtypes.bfloat16 if GEMM_BF16 else np.float32
    x = np.asarray(inputs["x"], dtype=np.float32)
    pos = np.asarray(inputs["pos"], dtype=np.float32)
    common = {}
    for n in ["qkv_w", "proj_w"]:
        common[n] = np.ascontiguousarray(np.asarray(inputs[n], np.float32).astype(wnp))
    f8np = ml_dtypes.float8_e4m3fn
    # ff_w1 fp8 DoubleRow layout: [g, p, ctp, j, mtj, m], ct = 2*ctp + j
    w1 = np.asarray(inputs["ff_w1"], np.float32).reshape(2, 2, P, W1G, 8, P)
    common["ff_w1"] = np.ascontiguousarray(
        w1.transpose(3, 2, 0, 1, 4, 5).reshape(W1G, P, CT * 8 * P).astype(f8np))
    # ff_w2 fp8 DoubleRow layout: [g, p, mtpg, jj, m], mt = 2*mtp + jj
    w2 = np.asarray(inputs["ff_w2"], np.float32).reshape(W1G, 4, 2, P, C)
    common["ff_w2"] = np.ascontiguousarray(
        w2.transpose(0, 3, 1, 2, 4).reshape(W1G, P, 8 * C).astype(f8np))
    for i in (1, 2):
        for n in (f"cp{i}_down_w", f"cp{i}_conv_w", f"cp{i}_up_w"):
            common[n] = np.ascontiguousarray(np.asarray(inputs[n], np.float32).astype(wnp))
    for n, flag in nz.items():
        if flag:
            common[n] = np.ascontiguousarray(
                np.asarray(inputs[n], np.float32)).reshape(-1, 1)
    in_maps = []
    for c in range(NCORES):
        m = dict(common)
        m["x"] = np.ascontiguousarray(
            x[c * BPC:(c + 1) * BPC].transpose(2, 0, 1).reshape(C, TOK))
        m["pos"] = np.ascontiguousarray(
            pos[c * BPC:(c + 1) * BPC].transpose(2, 0, 1).reshape(C, TOK))
        in_maps.append(m)
    return in_maps


def kernel(**inputs):
    nz = input_flags(inputs)
    nz_key = tuple(sorted((k, v) for k, v in nz.items()))
    nc = _get_compiled(nz_key, nz)
    in_maps = make_in_maps(inputs, nz)
    res = run_bass_kernel_spmd(nc, in_maps, core_ids=list(range(NCORES)))
    out = np.concatenate(
        [res.results[c]["out"].reshape(C, BPC, N).transpose(1, 2, 0)
         for c in range(NCORES)], axis=0)
    return np.ascontiguousarray(out.astype(np.float32))


if __name__ == "__main__":
    # quick self-build check (no run)
    nc = bacc.Bacc("TRN2", target_bir_lowering=False, debug=False, num_devices=NCORES)
    build(nc, {})
    nc.compile()
    print("built + compiled OK; instructions:",
          sum(len(bb.instructions) for bb in nc.main_func.blocks))
